# revision 7
# baseline (speedup 1.0000x reference)
"""Fused attention kernel for Trainium2, 8 NeuronCores.

Problem: B=4, T=2048, C=1024, nh=16, hs=64, fused QKV (chunk order k,q,v),
softmax attention, then (faithful reference bug) reshape (B,nh,T,hs)->(B,T,C)
directly before the output projection.

Key structural fact: with the buggy reshape, head h's attention output
occupies exactly rows [h*128, (h+1)*128) of the reshaped (T, C) matrix
(row tau = h*128 + t//16, col = (t%16)*64 + d). So everything after the
QKV projection is fully independent per (batch, head) pair; the output
projection needs no cross-head reduction.

Sharding: 8 cores = 4 batches x 2 head-groups (8 heads each). Each core
computes its batch's QKV slice and its 8 heads end-to-end. No collectives.

Schedule (v2): the Activation engine's exp stream (256 x [128,1024] tiles,
~1.04us each) is the pacing resource. Everything else is arranged so Act
starts early and never starves:
 - QKV is split into (mt, icx) subunits; only K0/Q0 run up front, the other
   six mt tiles are interleaved into the attention j-steps as PE fillers.
 - V tile units are emitted inside the first (hp0, ic0) j-loop just before
   the O matmul that consumes them.
 - Softmax denominators no longer stream exp tiles through the PE (the
   baseline's 512 ones-matmuls): the DVE tree-accumulates exp tiles in bf16
   (4x perf mode) and one gpsimd partition_all_reduce per (hp, ic) both
   reduces over keys and broadcasts across partitions. Reciprocal on DVE,
   normalization multiplies read the O psum directly.
 - Query blocks are strided (ic holds queries tau = ic mod 4) so each proj
   u-chunk (u = tau%16) depends on a single ic, letting the output
   projection drain during the attention stream instead of all at the end.
"""

import sys

import numpy as np

sys.path.insert(0, "/opt/trn_rl_repo")

import ml_dtypes  # noqa: E402

B, T, C = 4, 2048, 1024
NH, HS = 16, 64
NCORES = 8
HPC = 8  # heads per core

_CACHE = {}


def _build():
    from contextlib import ExitStack

    import concourse.bass as bass  # noqa: F401
    import concourse.bass_isa as bass_isa
    import concourse.mybir as mybir
    from concourse import bacc, library_config, tile

    F32 = mybir.dt.float32
    BF16 = mybir.dt.bfloat16
    ADD = mybir.AluOpType.add
    MULT = mybir.AluOpType.mult
    EXP = mybir.ActivationFunctionType.Exp
    RADD = bass_isa.ReduceOp.add

    nc = bacc.Bacc()
    xT = nc.dram_tensor("xT", [128, 8, 2048], BF16, kind="ExternalInput")
    wqkv = nc.dram_tensor("wqkv", [128, 8, 1536], BF16, kind="ExternalInput")
    bqk = nc.dram_tensor("bqk", [128, 8], F32, kind="ExternalInput")
    bv = nc.dram_tensor("bv", [128, 512], F32, kind="ExternalInput")
    wp = nc.dram_tensor("wp", [64, 16, 1024], BF16, kind="ExternalInput")
    pb = nc.dram_tensor("pb", [128, 1024], F32, kind="ExternalInput")
    y = nc.dram_tensor("y", [128, 8, 1024], F32, kind="ExternalOutput")

    MT_ORDER = [0, 4, 1, 5, 2, 6, 3, 7]

    with tile.TileContext(nc) as tc, ExitStack() as ctx:
        nc.gpsimd.load_library(library_config.attn)

        persist = ctx.enter_context(tc.tile_pool(name="persist", bufs=1))
        wsp = ctx.enter_context(tc.tile_pool(name="wstream", bufs=2))
        utp = ctx.enter_context(tc.tile_pool(name="utp", bufs=4))
        trp = ctx.enter_context(tc.tile_pool(name="treep", bufs=3))
        nrm = ctx.enter_context(tc.tile_pool(name="nrm", bufs=2))
        yps = ctx.enter_context(tc.tile_pool(name="ysb", bufs=2))

        # ---- persistent SBUF tensors + early DMAs --------------------
        bqk_sb = persist.tile([128, 8], F32, tag="bqk")
        nc.sync.dma_start(bqk_sb, bqk[:])
        bv_sb = persist.tile([128, 512], F32, tag="bv")
        nc.sync.dma_start(bv_sb, bv[:])
        pb_sb = persist.tile([128, 1024], F32, tag="pb")
        nc.sync.dma_start(pb_sb, pb[:])

        xts = persist.tile([128, 8, 2048], BF16, tag="xts")
        wt = {}

        def prefetch_wt(mt):
            wt[mt] = wsp.tile([128, 8, 128], BF16, tag="wt", name=f"wt{mt}")
            nc.sync.dma_start(wt[mt], wqkv[:, :, mt * 128:(mt + 1) * 128])

        prefetch_wt(MT_ORDER[0])
        nc.sync.dma_start(xts[:, :, 0:512], xT[:, :, 0:512])
        prefetch_wt(MT_ORDER[1])
        for q in range(1, 4):
            nc.sync.dma_start(xts[:, :, q * 512:(q + 1) * 512],
                              xT[:, :, q * 512:(q + 1) * 512])
        wv_sb = persist.tile([128, 8, 512], BF16, tag="wv")
        nc.sync.dma_start(wv_sb, wqkv[:, :, 1024:1536])
        wp_sb = persist.tile([128, 16, 1024], BF16, tag="wp")
        nc.sync.dma_start(wp_sb[0:64], wp[:])
        nc.sync.dma_start(wp_sb[64:128], wp[:])

        qk = [persist.tile([128, 2048], BF16, tag=f"qk{mt}", name=f"qk{mt}")
              for mt in range(8)]
        vbuf = persist.tile([128, 16, HPC, 64], BF16, tag="vbuf")
        ots = [persist.tile([128, 2048], BF16, tag=f"ot{hp}", name=f"ot{hp}")
               for hp in range(4)]
        # strided views: free index tau = t*4 + four; ic-chunk ic covers
        # queries tau with tau % 4 == ic
        qk_s = [t.rearrange("p (t four) -> p four t", four=4) for t in qk]
        ots_s = [t.rearrange("d (t four) -> d four t", four=4) for t in ots]

        # PSUM budget (8 banks): sp ring 2x[128,1024] = 4, O double-buffer
        # 2x[128,512] = 2, proj ypA+ypB = 2. QKV/V units borrow sp-ring
        # tiles for their accumulations instead of a dedicated bank.
        spx = ctx.enter_context(tc.tile_pool(name="spool", bufs=2, space="PSUM"))
        opx = ctx.enter_context(tc.tile_pool(name="opool", bufs=2, space="PSUM"))
        ypx = ctx.enter_context(tc.tile_pool(name="ypool", bufs=2, space="PSUM"))

        # ---- QKV / V unit emitters -----------------------------------
        def qkv_subunit(mt, icx):
            ps = spx.tile([128, 1024], F32, tag="sp", name="qkvps")
            isl = slice(icx * 512, (icx + 1) * 512)
            for ct in range(8):
                nc.tensor.matmul(ps[:, 0:512], wt[mt][:, ct, :],
                                 xts[:, ct, isl],
                                 start=(ct == 0), stop=(ct == 7))
            nc.vector.tensor_tensor(
                qk[mt][:, isl], ps[:, 0:512],
                bqk_sb[:, mt:mt + 1].to_broadcast((128, 512)), ADD)

        def v_unit(tt):
            ps = spx.tile([128, 1024], F32, tag="sp", name="vps")
            tsl = slice(tt * 128, (tt + 1) * 128)
            for ct in range(8):
                nc.tensor.matmul(ps[:, 0:512], xts[:, ct, tsl],
                                 wv_sb[:, ct, :],
                                 start=(ct == 0), stop=(ct == 7))
            nc.vector.tensor_tensor(
                vbuf[:, tt, :, :],
                ps[:, 0:512].rearrange("p (h d) -> p h d", d=64),
                bv_sb.rearrange("p (h d) -> p h d", d=64), ADD)

        # K0 / Q0 up front; the rest interleaves into the attention stream
        for icx in range(4):
            qkv_subunit(MT_ORDER[0], icx)
        for icx in range(4):
            qkv_subunit(MT_ORDER[1], icx)

        # mt tiles consumed as fillers during head-pair hp feed hp+1's S
        # matmuls, so they must fully drain before that hp block ends.
        qkv_pending = {hp: [(MT_ORDER[2 + 2 * hp], icx) for icx in range(4)]
                       + [(MT_ORDER[3 + 2 * hp], icx) for icx in range(4)]
                       for hp in range(3)}

        # ---- attention -----------------------------------------------
        def s_exp(hp, ic, j):
            kt = qk[hp]
            qts = qk_s[4 + hp]
            jsl = slice(j * 128, (j + 1) * 128)
            sp = spx.tile([128, 1024], F32, tag="sp", name="sp")
            nc.tensor.matmul(sp[:, 0:512], kt[0:64, jsl], qts[0:64, ic, :],
                             start=True, stop=True)
            nc.tensor.matmul(sp[:, 512:1024], kt[64:128, jsl],
                             qts[64:128, ic, :], start=True, stop=True)
            ut = utp.tile([128, 1024], BF16, tag="ut", name="ut")
            nc.scalar.activation(ut, sp, EXP, scale=0.125)
            return ut

        proj_state = {}
        proj_q = []

        def proj_mm(hp, q2, u, ustep):
            # row-packed pair: head A weights at array rows 0:64, head B at
            # 64:128; separate psum tiles accumulating over all 16 u.
            if "ypA" not in proj_state:
                proj_state["ypA"] = ypx.tile([128, 512], F32, tag="yp",
                                             name="ypA")
                proj_state["ypB"] = ypx.tile([128, 512], F32, tag="yp",
                                             name="ypB")
            ypA, ypB = proj_state["ypA"], proj_state["ypB"]
            otr = ots[hp].rearrange("d (t u) -> d u t", u=16)
            csl = slice(q2 * 512, (q2 + 1) * 512)
            nc.tensor.matmul(ypA, otr[0:64, u, :], wp_sb[0:64, u, csl],
                             start=(ustep == 0), stop=(ustep == 15))
            nc.tensor.matmul(ypB, otr[64:128, u, :], wp_sb[64:128, u, csl],
                             start=(ustep == 0), stop=(ustep == 15))
            if ustep == 15:
                for h, yp in ((2 * hp, ypA), (2 * hp + 1, ypB)):
                    ysb = yps.tile([128, 512], F32, tag="ysb", name="ysb")
                    nc.vector.tensor_tensor(ysb, yp, pb_sb[:, csl], ADD)
                    nc.sync.dma_start(y[:, h, csl], ysb)
                proj_state.clear()

        # per-(hp,q2) count of emitted proj u-steps (for start/stop flags)
        proj_ucnt = {}

        def drain_proj():
            if not proj_q:
                return
            hp, q2, u = proj_q.pop(0)
            ustep = proj_ucnt.get((hp, q2), 0)
            proj_ucnt[(hp, q2)] = ustep + 1
            proj_mm(hp, q2, u, ustep)

        seq = [(hp, ic, j) for hp in range(4) for ic in range(4)
               for j in range(16)]
        tree = {}

        pend = s_exp(*seq[0])
        optile = None
        for idx, (hp, ic, j) in enumerate(seq):
            hA, hB = 2 * hp, 2 * hp + 1
            if j == 0:
                optile = opx.tile([128, 512], F32, tag="op", name="op")
                if ic == 0 and hp <= 2:
                    # stream in the wt tiles for this hp's interleaved units
                    prefetch_wt(MT_ORDER[2 + 2 * hp])
                    prefetch_wt(MT_ORDER[3 + 2 * hp])
            nxt = s_exp(*seq[idx + 1]) if idx + 1 < len(seq) else None
            # fillers that must precede or may overlap the O pair
            if hp == 0 and ic == 0:
                v_unit(j)
            elif hp <= 2 and qkv_pending[hp] and (ic * 16 + j - 16) % 6 == 1:
                mt, icx = qkv_pending[hp].pop(0)
                qkv_subunit(mt, icx)
            # col-packed V pair: head A -> psum partitions 0:64, head B ->
            # 64:128, concurrent in the array
            nc.tensor.matmul(optile[0:64, :], vbuf[:, j, hA, :],
                             pend[:, 0:512], start=(j == 0), stop=(j == 15))
            nc.tensor.matmul(optile[64:128, :], vbuf[:, j, hB, :],
                             pend[:, 512:1024], start=(j == 0), stop=(j == 15),
                             tile_position=(0, 64))
            # denominator tree accumulation on DVE (bf16 4x mode)
            if j % 2 == 1:
                p = trp.tile([128, 1024], BF16, tag="tp", name="tp")
                nc.vector.tensor_tensor(p, tree.pop("u"), pend, ADD)
                tree[("p", (j // 2) % 2)] = p
            else:
                tree["u"] = pend
            if j % 4 == 3:
                qt_ = trp.tile([128, 1024], BF16, tag="tq", name="tq")
                nc.vector.tensor_tensor(qt_, tree.pop(("p", 0)),
                                        tree.pop(("p", 1)), ADD)
                tree[("q", (j // 4) % 2)] = qt_
            if j % 8 == 7:
                r = trp.tile([128, 1024], BF16, tag="tr", name="tr")
                nc.vector.tensor_tensor(r, tree.pop(("q", 0)),
                                        tree.pop(("q", 1)), ADD)
                tree[("r", j // 8)] = r
            pend = nxt
            drain_proj()
            if j == 15:
                accf = nrm.tile([128, 1024], F32, tag="accf", name="accf")
                nc.vector.tensor_tensor(accf, tree.pop(("r", 0)),
                                        tree.pop(("r", 1)), ADD)
                dbc = nrm.tile([128, 1024], F32, tag="dbc", name="dbc")
                nc.gpsimd.partition_all_reduce(dbc, accf, 128, RADD)
                rbc = nrm.tile([128, 1024], F32, tag="rbc", name="rbc")
                nc.vector.reciprocal(rbc, dbc)
                nc.vector.tensor_tensor(ots_s[hp][0:64, ic, :],
                                        optile[0:64, :], rbc[0:64, 0:512],
                                        MULT)
                nc.vector.tensor_tensor(ots_s[hp][64:128, ic, :],
                                        optile[64:128, :],
                                        rbc[64:128, 512:1024], MULT)
                # proj u-chunks whose tokens (tau%16 == u) live in this ic
                # (tau%4 == ic) are now complete: u in {ic, ic+4, ic+8, ic+12}.
                # Only one (hp, q2) accumulation group may be open at a time
                # (one ypA/ypB pair), so q2=0 fills as ics complete and q2=1
                # queues all at once when the head pair is done.
                proj_q.extend([(hp, 0, u) for u in range(ic, 16, 4)])
                if ic == 3:
                    proj_q.extend([(hp, 1, u) for u in range(16)])
        while proj_q:
            drain_proj()

    nc.compile()
    return nc


def _in_maps(x, w_weight, w_bias, proj_weight, proj_bias):
    x = np.ascontiguousarray(x, np.float32)
    w_weight = np.ascontiguousarray(w_weight, np.float32)
    w_bias = np.ascontiguousarray(w_bias, np.float32)
    proj_weight = np.ascontiguousarray(proj_weight, np.float32)
    proj_bias = np.ascontiguousarray(proj_bias, np.float32)

    wpT = np.ascontiguousarray(
        proj_weight.T.reshape(16, 64, 1024).transpose(1, 0, 2).astype(ml_dtypes.bfloat16))
    pbr = np.ascontiguousarray(np.tile(proj_bias[None], (128, 1)))

    maps = []
    for c in range(NCORES):
        b = c // 2
        h0 = (c % 2) * HPC
        xTc = np.ascontiguousarray(
            x[b].T.reshape(8, 128, 2048).transpose(1, 0, 2).astype(ml_dtypes.bfloat16))
        wk = w_weight[h0 * 64: h0 * 64 + 512]
        wq = w_weight[1024 + h0 * 64: 1024 + h0 * 64 + 512]
        wv = w_weight[2048 + h0 * 64: 2048 + h0 * 64 + 512]
        wqkvT = np.concatenate([wk.T, wq.T, wv.T], axis=1)  # [1024, 1536]
        wqkvT = np.ascontiguousarray(
            wqkvT.reshape(8, 128, 1536).transpose(1, 0, 2).astype(ml_dtypes.bfloat16))
        bk = w_bias[h0 * 64: h0 * 64 + 512]
        bq = w_bias[1024 + h0 * 64: 1024 + h0 * 64 + 512]
        bvc = w_bias[2048 + h0 * 64: 2048 + h0 * 64 + 512]
        bqkc = np.ascontiguousarray(
            np.concatenate([bk.reshape(4, 128).T, bq.reshape(4, 128).T], axis=1))
        bvr = np.ascontiguousarray(np.tile(bvc[None], (128, 1)))
        maps.append({
            "xT": xTc, "wqkv": wqkvT, "bqk": bqkc, "bv": bvr,
            "wp": wpT, "pb": pbr,
        })
    return maps


def _install_ntff_hook():
    """Register the axon NTFF profiling hook (missing antenv.axon_hooks shim)."""
    import contextlib
    import ctypes
    import types

    if "antenv.axon_hooks" in sys.modules:
        return
    import antenv
    so_path = "/opt/axon/libaxon_pjrt.so"
    try:
        lib = ctypes.CDLL(so_path)
    except OSError:
        return
    if not hasattr(lib, "axon_start_nrt_profile"):
        return
    lib.axon_start_nrt_profile.argtypes = [ctypes.POINTER(ctypes.c_int64),
                                           ctypes.c_size_t]
    lib.axon_start_nrt_profile.restype = ctypes.c_int64
    lib.axon_stop_nrt_profile.argtypes = [ctypes.c_char_p]
    lib.axon_stop_nrt_profile.restype = ctypes.c_int64

    @contextlib.contextmanager
    def _hook(output_dir, device_ids):
        import jax
        jax.devices()
        if device_ids:
            ids = (ctypes.c_int64 * len(device_ids))(*device_ids)
            rc = lib.axon_start_nrt_profile(ids, len(device_ids))
        else:
            rc = lib.axon_start_nrt_profile(None, 0)
        if rc != 0:
            raise RuntimeError(f"axon_start_nrt_profile rc={rc}")
        try:
            yield
        finally:
            n = lib.axon_stop_nrt_profile(str(output_dir).encode())
            print(f"profile: {n} file(s) written to {output_dir}", file=sys.stderr)

    mod = types.ModuleType("antenv.axon_hooks")
    mod.get_axon_ntff_profile_hook = lambda: _hook
    mod.set_axon_ntff_profile_hook = lambda h: None
    sys.modules["antenv.axon_hooks"] = mod
    antenv.axon_hooks = mod


def _run(x, w_weight, w_bias, proj_weight, proj_bias, trace=False):
    from concourse.bass_utils import run_bass_kernel_spmd

    if trace:
        _install_ntff_hook()

    if "nc" not in _CACHE:
        _CACHE["nc"] = _build()
    nc = _CACHE["nc"]
    maps = _in_maps(x, w_weight, w_bias, proj_weight, proj_bias)
    res = run_bass_kernel_spmd(nc, maps, core_ids=list(range(NCORES)), trace=trace)
    out = np.zeros((B, T, C), np.float32)
    for c in range(NCORES):
        yc = res.results[c]["y"]  # [128, 8, 1024]
        b = c // 2
        h0 = (c % 2) * HPC
        for j in range(HPC):
            out[b, (h0 + j) * 128:(h0 + j + 1) * 128, :] = yc[:, j, :]
    return out, res.exec_time_ns


def kernel(x, w_weight, w_bias, proj_weight, proj_bias):
    out, _ = _run(x, w_weight, w_bias, proj_weight, proj_bias, trace=False)
    return out


def kernel_with_time(x, w_weight, w_bias, proj_weight, proj_bias):
    return _run(x, w_weight, w_bias, proj_weight, proj_bias, trace=True)


# revision 14
# speedup vs baseline: 1.3982x; 1.3982x over previous
"""Fused attention kernel for Trainium2, 8 NeuronCores.

Problem: B=4, T=2048, C=1024, nh=16, hs=64, fused QKV (chunk order k,q,v),
softmax attention, then (faithful reference bug) reshape (B,nh,T,hs)->(B,T,C)
directly before the output projection.

Key structural fact: with the buggy reshape, head h's attention output
occupies exactly rows [h*128, (h+1)*128) of the reshaped (T, C) matrix
(row tau = h*128 + t//16, col = (t%16)*64 + d). So everything after the
QKV projection is fully independent per (batch, head) pair; the output
projection needs no cross-head reduction.

Sharding: 8 cores = 4 batches x 2 head-groups (8 heads each). Each core
computes its batch's QKV slice and its 8 heads end-to-end. No collectives.

Schedule (v2): the Activation engine's exp stream (256 x [128,1024] tiles,
~1.04us each) is the pacing resource. Everything else is arranged so Act
starts early and never starves:
 - QKV is split into (mt, icx) subunits; only K0/Q0 run up front, the other
   six mt tiles are interleaved into the attention j-steps as PE fillers.
 - V tile units are emitted inside the first (hp0, ic0) j-loop just before
   the O matmul that consumes them.
 - Softmax denominators no longer stream exp tiles through the PE (the
   baseline's 512 ones-matmuls): the DVE tree-accumulates exp tiles in bf16
   (4x perf mode) and one gpsimd partition_all_reduce per (hp, ic) both
   reduces over keys and broadcasts across partitions. Reciprocal on DVE,
   normalization multiplies read the O psum directly.
 - Query blocks are strided (ic holds queries tau = ic mod 4) so each proj
   u-chunk (u = tau%16) depends on a single ic, letting the output
   projection drain during the attention stream instead of all at the end.
"""

import sys

import numpy as np

sys.path.insert(0, "/opt/trn_rl_repo")

import ml_dtypes  # noqa: E402

B, T, C = 4, 2048, 1024
NH, HS = 16, 64
NCORES = 8
HPC = 8  # heads per core

_CACHE = {}


def _build():
    from contextlib import ExitStack

    import concourse.bass as bass  # noqa: F401
    import concourse.bass_isa as bass_isa
    import concourse.mybir as mybir
    from concourse import bacc, library_config, tile

    F32 = mybir.dt.float32
    BF16 = mybir.dt.bfloat16
    ADD = mybir.AluOpType.add
    MULT = mybir.AluOpType.mult
    EXP = mybir.ActivationFunctionType.Exp

    nc = bacc.Bacc()
    vones = nc.dram_tensor("vones", [128, 1], BF16, kind="ExternalInput")
    xT = nc.dram_tensor("xT", [128, 8, 2048], BF16, kind="ExternalInput")
    wqkv = nc.dram_tensor("wqkv", [128, 8, 1536], BF16, kind="ExternalInput")
    bqk = nc.dram_tensor("bqk", [128, 8], F32, kind="ExternalInput")
    bv = nc.dram_tensor("bv", [128, 512], F32, kind="ExternalInput")
    wp = nc.dram_tensor("wp", [64, 16, 1024], BF16, kind="ExternalInput")
    pb = nc.dram_tensor("pb", [128, 1024], F32, kind="ExternalInput")
    y = nc.dram_tensor("y", [128, 8, 1024], F32, kind="ExternalOutput")

    MT_ORDER = [0, 4, 1, 5, 2, 6, 3, 7]

    with tile.TileContext(nc) as tc, ExitStack() as ctx:
        persist = ctx.enter_context(tc.tile_pool(name="persist", bufs=1))
        wsp = ctx.enter_context(tc.tile_pool(name="wstream", bufs=2))
        utp = ctx.enter_context(tc.tile_pool(name="utp", bufs=4))
        trp = ctx.enter_context(tc.tile_pool(name="treep", bufs=3))
        nrm = ctx.enter_context(tc.tile_pool(name="nrm", bufs=2))
        yps = ctx.enter_context(tc.tile_pool(name="ysb", bufs=2))

        # ---- persistent SBUF tensors + early DMAs --------------------
        bqk_sb = persist.tile([128, 8], F32, tag="bqk")
        nc.sync.dma_start(bqk_sb, bqk[:])
        bv_sb = persist.tile([128, 512], F32, tag="bv")
        nc.sync.dma_start(bv_sb, bv[:])
        pb_sb = persist.tile([128, 1024], F32, tag="pb")
        nc.sync.dma_start(pb_sb, pb[:])
        ones128_sb = persist.tile([128, 1], BF16, tag="ones128")
        nc.sync.dma_start(ones128_sb, vones[:])

        xts = persist.tile([128, 8, 2048], BF16, tag="xts")
        wt = {}

        def prefetch_wt(mt):
            wt[mt] = wsp.tile([128, 8, 128], BF16, tag="wt", name=f"wt{mt}")
            nc.sync.dma_start(wt[mt], wqkv[:, :, mt * 128:(mt + 1) * 128])

        prefetch_wt(MT_ORDER[0])
        nc.sync.dma_start(xts[:, :, 0:512], xT[:, :, 0:512])
        prefetch_wt(MT_ORDER[1])
        for q in range(1, 4):
            nc.sync.dma_start(xts[:, :, q * 512:(q + 1) * 512],
                              xT[:, :, q * 512:(q + 1) * 512])
        wv_sb = persist.tile([128, 8, 512], BF16, tag="wv")
        nc.sync.dma_start(wv_sb, wqkv[:, :, 1024:1536])
        wp_sb = persist.tile([128, 16, 1024], BF16, tag="wp")
        nc.sync.dma_start(wp_sb[0:64], wp[:])
        nc.sync.dma_start(wp_sb[64:128], wp[:])

        qk = [persist.tile([128, 2048], BF16, tag=f"qk{mt}", name=f"qk{mt}")
              for mt in range(8)]
        vbuf = persist.tile([128, 16, HPC, 64], BF16, tag="vbuf")
        ots = [persist.tile([128, 2048], BF16, tag=f"ot{hp}", name=f"ot{hp}")
               for hp in range(4)]
        # strided views: free index tau = t*4 + four; ic-chunk ic covers
        # queries tau with tau % 4 == ic
        qk_s = [t.rearrange("p (t four) -> p four t", four=4) for t in qk]
        ots_s = [t.rearrange("d (t four) -> d four t", four=4) for t in ots]

        # PSUM budget (8 banks): sp ring 2x[128,1024] = 4, O accumulate 1,
        # proj ypA+ypB = 2, QKV/V-unit accumulator 1. The per-(hp,ic)
        # denominator ones-matmul output borrows an sp-ring tile.
        spx = ctx.enter_context(tc.tile_pool(name="spool", bufs=2, space="PSUM"))
        opx = ctx.enter_context(tc.tile_pool(name="opool", bufs=1, space="PSUM"))
        ypx = ctx.enter_context(tc.tile_pool(name="ypool", bufs=2, space="PSUM"))
        aux = ctx.enter_context(tc.tile_pool(name="auxp", bufs=1, space="PSUM"))
        dpool = ctx.enter_context(tc.tile_pool(name="dpool", bufs=2,
                                               space="DRAM"))

        # ---- QKV / V unit emitters -----------------------------------
        def qkv_subunit(mt, icx):
            ps = aux.tile([128, 512], F32, tag="qkvps", name="qkvps")
            isl = slice(icx * 512, (icx + 1) * 512)
            for ct in range(8):
                nc.tensor.matmul(ps, wt[mt][:, ct, :], xts[:, ct, isl],
                                 start=(ct == 0), stop=(ct == 7))
            nc.vector.tensor_tensor(
                qk[mt][:, isl], ps,
                bqk_sb[:, mt:mt + 1].to_broadcast((128, 512)), ADD)

        def v_unit(tt):
            ps = aux.tile([128, 512], F32, tag="qkvps", name="vps")
            tsl = slice(tt * 128, (tt + 1) * 128)
            for ct in range(8):
                nc.tensor.matmul(ps, xts[:, ct, tsl], wv_sb[:, ct, :],
                                 start=(ct == 0), stop=(ct == 7))
            nc.vector.tensor_tensor(
                vbuf[:, tt, :, :],
                ps.rearrange("p (h d) -> p h d", d=64),
                bv_sb.rearrange("p (h d) -> p h d", d=64), ADD)

        # K0 / Q0 up front; the rest interleaves into the attention stream
        for icx in range(4):
            qkv_subunit(MT_ORDER[0], icx)
        for icx in range(4):
            qkv_subunit(MT_ORDER[1], icx)

        # mt tiles consumed as fillers during head-pair hp feed hp+1's S
        # matmuls, so they must fully drain before that hp block ends.
        qkv_pending = {hp: [(MT_ORDER[2 + 2 * hp], icx) for icx in range(4)]
                       + [(MT_ORDER[3 + 2 * hp], icx) for icx in range(4)]
                       for hp in range(3)}

        # ---- attention -----------------------------------------------
        def s_exp(hp, ic, j):
            kt = qk[hp]
            qts = qk_s[4 + hp]
            jsl = slice(j * 128, (j + 1) * 128)
            sp = spx.tile([128, 1024], F32, tag="sp", name="sp")
            nc.tensor.matmul(sp[:, 0:512], kt[0:64, jsl], qts[0:64, ic, :],
                             start=True, stop=True)
            nc.tensor.matmul(sp[:, 512:1024], kt[64:128, jsl],
                             qts[64:128, ic, :], start=True, stop=True)
            ut = utp.tile([128, 1024], BF16, tag="ut", name="ut")
            nc.scalar.activation(ut, sp, EXP, scale=0.125)
            return ut

        proj_state = {}
        proj_q = []

        def proj_mm(hp, q2, u, ustep):
            # row-packed pair: head A weights at array rows 0:64, head B at
            # 64:128; separate psum tiles accumulating over all 16 u.
            if "ypA" not in proj_state:
                proj_state["ypA"] = ypx.tile([128, 512], F32, tag="yp",
                                             name="ypA")
                proj_state["ypB"] = ypx.tile([128, 512], F32, tag="yp",
                                             name="ypB")
            ypA, ypB = proj_state["ypA"], proj_state["ypB"]
            otr = ots[hp].rearrange("d (t u) -> d u t", u=16)
            csl = slice(q2 * 512, (q2 + 1) * 512)
            nc.tensor.matmul(ypA, otr[0:64, u, :], wp_sb[0:64, u, csl],
                             start=(ustep == 0), stop=(ustep == 15))
            nc.tensor.matmul(ypB, otr[64:128, u, :], wp_sb[64:128, u, csl],
                             start=(ustep == 0), stop=(ustep == 15))
            if ustep == 15:
                for h, yp in ((2 * hp, ypA), (2 * hp + 1, ypB)):
                    ysb = yps.tile([128, 512], F32, tag="ysb", name="ysb")
                    nc.vector.tensor_tensor(ysb, yp, pb_sb[:, csl], ADD)
                    nc.sync.dma_start(y[:, h, csl], ysb)
                proj_state.clear()

        # per-(hp,q2) count of emitted proj u-steps (for start/stop flags)
        proj_ucnt = {}

        def drain_proj():
            if not proj_q:
                return
            hp, q2, u = proj_q.pop(0)
            ustep = proj_ucnt.get((hp, q2), 0)
            proj_ucnt[(hp, q2)] = ustep + 1
            proj_mm(hp, q2, u, ustep)

        seq = [(hp, ic, j) for hp in range(4) for ic in range(4)
               for j in range(16)]
        tree = {}

        pend = s_exp(*seq[0])
        optile = None
        for idx, (hp, ic, j) in enumerate(seq):
            hA, hB = 2 * hp, 2 * hp + 1
            if j == 0:
                optile = opx.tile([128, 512], F32, tag="op", name="op")
                if ic == 0 and hp <= 2:
                    # stream in the wt tiles for this hp's interleaved units
                    prefetch_wt(MT_ORDER[2 + 2 * hp])
                    prefetch_wt(MT_ORDER[3 + 2 * hp])
            nxt = s_exp(*seq[idx + 1]) if idx + 1 < len(seq) else None
            # fillers that must precede or may overlap the O pair
            if hp == 0 and ic == 0:
                v_unit(j)
            elif hp <= 2 and qkv_pending[hp] and (ic * 16 + j - 16) % 6 == 1:
                mt, icx = qkv_pending[hp].pop(0)
                qkv_subunit(mt, icx)
            # col-packed V pair: head A -> psum partitions 0:64, head B ->
            # 64:128, concurrent in the array
            nc.tensor.matmul(optile[0:64, :], vbuf[:, j, hA, :],
                             pend[:, 0:512], start=(j == 0), stop=(j == 15))
            nc.tensor.matmul(optile[64:128, :], vbuf[:, j, hB, :],
                             pend[:, 512:1024], start=(j == 0), stop=(j == 15),
                             tile_position=(0, 64))
            # denominator tree accumulation on DVE (bf16 4x mode)
            if j % 2 == 1:
                p = trp.tile([128, 1024], BF16, tag="tp", name="tp")
                nc.vector.tensor_tensor(p, tree.pop("u"), pend, ADD)
                tree[("p", (j // 2) % 2)] = p
            else:
                tree["u"] = pend
            if j % 4 == 3:
                qt_ = trp.tile([128, 1024], BF16, tag="tq", name="tq")
                nc.vector.tensor_tensor(qt_, tree.pop(("p", 0)),
                                        tree.pop(("p", 1)), ADD)
                tree[("q", (j // 4) % 2)] = qt_
            if j % 8 == 7:
                r = trp.tile([128, 1024], BF16, tag="tr", name="tr")
                nc.vector.tensor_tensor(r, tree.pop(("q", 0)),
                                        tree.pop(("q", 1)), ADD)
                tree[("r", j // 8)] = r
            pend = nxt
            drain_proj()
            if j == 15:
                accf = trp.tile([128, 1024], BF16, tag="ta", name="accf")
                nc.vector.tensor_tensor(accf, tree.pop(("r", 0)),
                                        tree.pop(("r", 1)), ADD)
                # copy the O psum out right away so the next ic's O chain
                # can start; normalize on the SBUF copy.
                osb = nrm.tile([128, 512], F32, tag="osb", name="osb")
                nc.vector.tensor_copy(osb, optile)
                # denominators: one col-packed ones-matmul pair over the
                # tree-accumulated exp sums (head A -> psum row 0, B -> 32)
                rsps = spx.tile([128, 1024], F32, tag="sp", name="rsps")
                nc.tensor.matmul(rsps[0:1, 0:512], ones128_sb,
                                 accf[:, 0:512], start=True, stop=True)
                nc.tensor.matmul(rsps[32:33, 0:512], ones128_sb,
                                 accf[:, 512:1024], start=True, stop=True,
                                 tile_position=(0, 32))
                rsb = nrm.tile([33, 512], F32, tag="rsb", name="rsb")
                nc.vector.tensor_copy(rsb, rsps[0:33, 0:512])
                # denominators for A and B -> DRAM as one [1024] row,
                # reciprocal at 8 els/lane, broadcast back into the
                # matching partition halves.
                scr1 = dpool.tile([1024], F32, tag="scr1", name="scr1")
                nc.sync.dma_start(
                    scr1.rearrange("(r f) -> r f", r=2), rsb[0:33:32, :])
                rst = nrm.tile([128, 8], F32, tag="rst", name="rst")
                nc.sync.dma_start(rst, scr1.rearrange("(p f) -> p f", f=8))
                nc.vector.reciprocal(rst, rst)
                scr2 = dpool.tile([1024], F32, tag="scr2", name="scr2")
                nc.sync.dma_start(scr2.rearrange("(p f) -> p f", f=8), rst)
                bcsb = nrm.tile([128, 512], F32, tag="bcsb", name="bcsb")
                nc.sync.dma_start(
                    bcsb[0:64, :], scr2[None, 0:512].to_broadcast((64, 512)))
                nc.sync.dma_start(
                    bcsb[64:128, :],
                    scr2[None, 512:1024].to_broadcast((64, 512)))
                nc.vector.tensor_tensor(ots_s[hp][:, ic, :], osb, bcsb, MULT)
                # proj u-chunks whose tokens (tau%16 == u) live in this ic
                # (tau%4 == ic) are now complete: u in {ic, ic+4, ic+8, ic+12}.
                # Only one (hp, q2) accumulation group may be open at a time
                # (one ypA/ypB pair), so q2=0 fills as ics complete and q2=1
                # queues all at once when the head pair is done.
                proj_q.extend([(hp, 0, u) for u in range(ic, 16, 4)])
                if ic == 3:
                    proj_q.extend([(hp, 1, u) for u in range(16)])
        while proj_q:
            drain_proj()

    nc.compile()
    return nc


def _in_maps(x, w_weight, w_bias, proj_weight, proj_bias):
    x = np.ascontiguousarray(x, np.float32)
    w_weight = np.ascontiguousarray(w_weight, np.float32)
    w_bias = np.ascontiguousarray(w_bias, np.float32)
    proj_weight = np.ascontiguousarray(proj_weight, np.float32)
    proj_bias = np.ascontiguousarray(proj_bias, np.float32)

    wpT = np.ascontiguousarray(
        proj_weight.T.reshape(16, 64, 1024).transpose(1, 0, 2).astype(ml_dtypes.bfloat16))
    pbr = np.ascontiguousarray(np.tile(proj_bias[None], (128, 1)))
    vones = np.ones((128, 1), dtype=ml_dtypes.bfloat16)

    maps = []
    for c in range(NCORES):
        b = c // 2
        h0 = (c % 2) * HPC
        xTc = np.ascontiguousarray(
            x[b].T.reshape(8, 128, 2048).transpose(1, 0, 2).astype(ml_dtypes.bfloat16))
        wk = w_weight[h0 * 64: h0 * 64 + 512]
        wq = w_weight[1024 + h0 * 64: 1024 + h0 * 64 + 512]
        wv = w_weight[2048 + h0 * 64: 2048 + h0 * 64 + 512]
        wqkvT = np.concatenate([wk.T, wq.T, wv.T], axis=1)  # [1024, 1536]
        wqkvT = np.ascontiguousarray(
            wqkvT.reshape(8, 128, 1536).transpose(1, 0, 2).astype(ml_dtypes.bfloat16))
        bk = w_bias[h0 * 64: h0 * 64 + 512]
        bq = w_bias[1024 + h0 * 64: 1024 + h0 * 64 + 512]
        bvc = w_bias[2048 + h0 * 64: 2048 + h0 * 64 + 512]
        bqkc = np.ascontiguousarray(
            np.concatenate([bk.reshape(4, 128).T, bq.reshape(4, 128).T], axis=1))
        bvr = np.ascontiguousarray(np.tile(bvc[None], (128, 1)))
        maps.append({
            "xT": xTc, "wqkv": wqkvT, "bqk": bqkc, "bv": bvr,
            "wp": wpT, "pb": pbr, "vones": vones,
        })
    return maps


def _install_ntff_hook():
    """Register the axon NTFF profiling hook (missing antenv.axon_hooks shim)."""
    import contextlib
    import ctypes
    import types

    if "antenv.axon_hooks" in sys.modules:
        return
    import antenv
    so_path = "/opt/axon/libaxon_pjrt.so"
    try:
        lib = ctypes.CDLL(so_path)
    except OSError:
        return
    if not hasattr(lib, "axon_start_nrt_profile"):
        return
    lib.axon_start_nrt_profile.argtypes = [ctypes.POINTER(ctypes.c_int64),
                                           ctypes.c_size_t]
    lib.axon_start_nrt_profile.restype = ctypes.c_int64
    lib.axon_stop_nrt_profile.argtypes = [ctypes.c_char_p]
    lib.axon_stop_nrt_profile.restype = ctypes.c_int64

    @contextlib.contextmanager
    def _hook(output_dir, device_ids):
        import jax
        jax.devices()
        if device_ids:
            ids = (ctypes.c_int64 * len(device_ids))(*device_ids)
            rc = lib.axon_start_nrt_profile(ids, len(device_ids))
        else:
            rc = lib.axon_start_nrt_profile(None, 0)
        if rc != 0:
            raise RuntimeError(f"axon_start_nrt_profile rc={rc}")
        try:
            yield
        finally:
            n = lib.axon_stop_nrt_profile(str(output_dir).encode())
            print(f"profile: {n} file(s) written to {output_dir}", file=sys.stderr)

    mod = types.ModuleType("antenv.axon_hooks")
    mod.get_axon_ntff_profile_hook = lambda: _hook
    mod.set_axon_ntff_profile_hook = lambda h: None
    sys.modules["antenv.axon_hooks"] = mod
    antenv.axon_hooks = mod


def _run(x, w_weight, w_bias, proj_weight, proj_bias, trace=False):
    from concourse.bass_utils import run_bass_kernel_spmd

    if trace:
        _install_ntff_hook()

    if "nc" not in _CACHE:
        _CACHE["nc"] = _build()
    nc = _CACHE["nc"]
    maps = _in_maps(x, w_weight, w_bias, proj_weight, proj_bias)
    res = run_bass_kernel_spmd(nc, maps, core_ids=list(range(NCORES)), trace=trace)
    out = np.zeros((B, T, C), np.float32)
    for c in range(NCORES):
        yc = res.results[c]["y"]  # [128, 8, 1024]
        b = c // 2
        h0 = (c % 2) * HPC
        for j in range(HPC):
            out[b, (h0 + j) * 128:(h0 + j + 1) * 128, :] = yc[:, j, :]
    return out, res.exec_time_ns


def kernel(x, w_weight, w_bias, proj_weight, proj_bias):
    out, _ = _run(x, w_weight, w_bias, proj_weight, proj_bias, trace=False)
    return out


def kernel_with_time(x, w_weight, w_bias, proj_weight, proj_bias):
    return _run(x, w_weight, w_bias, proj_weight, proj_bias, trace=True)


# revision 18
# speedup vs baseline: 1.6485x; 1.1791x over previous
"""Fused attention kernel for Trainium2, 8 NeuronCores.

Problem: B=4, T=2048, C=1024, nh=16, hs=64, fused QKV (chunk order k,q,v),
softmax attention, then (faithful reference bug) reshape (B,nh,T,hs)->(B,T,C)
directly before the output projection.

Key structural fact: with the buggy reshape, head h's attention output
occupies exactly rows [h*128, (h+1)*128) of the reshaped (T, C) matrix
(row tau = h*128 + t//16, col = (t%16)*64 + d). So everything after the
QKV projection is fully independent per (batch, head) pair; the output
projection needs no cross-head reduction.

Sharding: 8 cores = 4 batches x 2 head-groups (8 heads each). Each core
computes its batch's QKV slice and its 8 heads end-to-end. No collectives.

Schedule (v2): the Activation engine's exp stream (256 x [128,1024] tiles,
~1.04us each) is the pacing resource. Everything else is arranged so Act
starts early and never starves:
 - QKV is split into (mt, icx) subunits; only K0/Q0 run up front, the other
   six mt tiles are interleaved into the attention j-steps as PE fillers.
 - V tile units are emitted inside the first (hp0, ic0) j-loop just before
   the O matmul that consumes them.
 - Softmax denominators no longer stream exp tiles through the PE (the
   baseline's 512 ones-matmuls): the DVE tree-accumulates exp tiles in bf16
   (4x perf mode) and one gpsimd partition_all_reduce per (hp, ic) both
   reduces over keys and broadcasts across partitions. Reciprocal on DVE,
   normalization multiplies read the O psum directly.
 - Query blocks are strided (ic holds queries tau = ic mod 4) so each proj
   u-chunk (u = tau%16) depends on a single ic, letting the output
   projection drain during the attention stream instead of all at the end.
"""

import sys

import numpy as np

sys.path.insert(0, "/opt/trn_rl_repo")

import ml_dtypes  # noqa: E402

B, T, C = 4, 2048, 1024
NH, HS = 16, 64
NCORES = 8
HPC = 8  # heads per core

_CACHE = {}


def _build():
    from contextlib import ExitStack

    import concourse.bass as bass  # noqa: F401
    import concourse.bass_isa as bass_isa
    import concourse.mybir as mybir
    from concourse import bacc, library_config, tile

    F32 = mybir.dt.float32
    BF16 = mybir.dt.bfloat16
    ADD = mybir.AluOpType.add
    MULT = mybir.AluOpType.mult
    EXP = mybir.ActivationFunctionType.Exp

    nc = bacc.Bacc()
    vones = nc.dram_tensor("vones", [128, 1], BF16, kind="ExternalInput")
    xT = nc.dram_tensor("xT", [128, 8, 2048], BF16, kind="ExternalInput")
    wqkv = nc.dram_tensor("wqkv", [128, 8, 1536], BF16, kind="ExternalInput")
    bqk = nc.dram_tensor("bqk", [128, 8], F32, kind="ExternalInput")
    bv = nc.dram_tensor("bv", [128, 512], F32, kind="ExternalInput")
    wp = nc.dram_tensor("wp", [64, 16, 1024], BF16, kind="ExternalInput")
    pb = nc.dram_tensor("pb", [128, 1024], F32, kind="ExternalInput")
    y = nc.dram_tensor("y", [128, 8, 1024], F32, kind="ExternalOutput")

    MT_ORDER = [0, 4, 1, 5, 2, 6, 3, 7]

    with tile.TileContext(nc) as tc, ExitStack() as ctx:
        persist = ctx.enter_context(tc.tile_pool(name="persist", bufs=1))
        wsp = ctx.enter_context(tc.tile_pool(name="wstream", bufs=2))
        utp = ctx.enter_context(tc.tile_pool(name="utp", bufs=4))
        trp = ctx.enter_context(tc.tile_pool(name="treep", bufs=3))
        nrm = ctx.enter_context(tc.tile_pool(name="nrm", bufs=2))
        yps = ctx.enter_context(tc.tile_pool(name="ysb", bufs=2))

        # ---- persistent SBUF tensors + early DMAs --------------------
        bqk_sb = persist.tile([128, 8], F32, tag="bqk")
        nc.sync.dma_start(bqk_sb, bqk[:])
        bv_sb = persist.tile([128, 512], F32, tag="bv")
        nc.sync.dma_start(bv_sb, bv[:])
        pb_sb = persist.tile([128, 1024], F32, tag="pb")
        nc.sync.dma_start(pb_sb, pb[:])
        ones128_sb = persist.tile([128, 1], BF16, tag="ones128")
        nc.sync.dma_start(ones128_sb, vones[:])

        xts = persist.tile([128, 8, 2048], BF16, tag="xts")
        wt = {}

        def prefetch_wt(mt):
            wt[mt] = wsp.tile([128, 8, 128], BF16, tag="wt", name=f"wt{mt}")
            nc.sync.dma_start(wt[mt], wqkv[:, :, mt * 128:(mt + 1) * 128])

        prefetch_wt(MT_ORDER[0])
        nc.sync.dma_start(xts[:, :, 0:512], xT[:, :, 0:512])
        prefetch_wt(MT_ORDER[1])
        for q in range(1, 4):
            nc.sync.dma_start(xts[:, :, q * 512:(q + 1) * 512],
                              xT[:, :, q * 512:(q + 1) * 512])
        wv_sb = persist.tile([128, 8, 512], BF16, tag="wv")
        nc.sync.dma_start(wv_sb, wqkv[:, :, 1024:1536])
        wp_sb = persist.tile([128, 16, 1024], BF16, tag="wp")
        nc.sync.dma_start(wp_sb[0:64], wp[:])
        nc.sync.dma_start(wp_sb[64:128], wp[:])

        qk = [persist.tile([128, 2048], BF16, tag=f"qk{mt}", name=f"qk{mt}")
              for mt in range(8)]
        vbuf = persist.tile([128, 16, HPC, 64], BF16, tag="vbuf")
        ots = [persist.tile([128, 2048], BF16, tag=f"ot{hp}", name=f"ot{hp}")
               for hp in range(4)]

        # PSUM budget (8 banks): sp ring 2x[128,1024] = 4, O accumulate 1,
        # proj ypA+ypB = 2, QKV/V-unit accumulator 1. The per-(hp,ic)
        # denominator ones-matmul output borrows an sp-ring tile.
        spx = ctx.enter_context(tc.tile_pool(name="spool", bufs=2, space="PSUM"))
        opx = ctx.enter_context(tc.tile_pool(name="opool", bufs=1, space="PSUM"))
        ypx = ctx.enter_context(tc.tile_pool(name="ypool", bufs=2, space="PSUM"))
        aux = ctx.enter_context(tc.tile_pool(name="auxp", bufs=1, space="PSUM"))
        dpool = ctx.enter_context(tc.tile_pool(name="dpool", bufs=2,
                                               space="DRAM"))

        # ---- QKV / V unit emitters -----------------------------------
        def qkv_subunit(mt, icx):
            ps = aux.tile([128, 512], F32, tag="qkvps", name="qkvps")
            isl = slice(icx * 512, (icx + 1) * 512)
            for ct in range(8):
                nc.tensor.matmul(ps, wt[mt][:, ct, :], xts[:, ct, isl],
                                 start=(ct == 0), stop=(ct == 7))
            nc.vector.tensor_tensor(
                qk[mt][:, isl], ps,
                bqk_sb[:, mt:mt + 1].to_broadcast((128, 512)), ADD)

        def v_unit(tt):
            ps = aux.tile([128, 512], F32, tag="qkvps", name="vps")
            tsl = slice(tt * 128, (tt + 1) * 128)
            for ct in range(8):
                nc.tensor.matmul(ps, xts[:, ct, tsl], wv_sb[:, ct, :],
                                 start=(ct == 0), stop=(ct == 7))
            nc.vector.tensor_tensor(
                vbuf[:, tt, :, :],
                ps.rearrange("p (h d) -> p h d", d=64),
                bv_sb.rearrange("p (h d) -> p h d", d=64), ADD)

        # Minimal warm-up: only the K0/Q0 chunks covering the first key
        # tiles and the first query block; everything else (6 subunits of
        # mt0/mt4, all 16 V units) packs into the (hp0, ic0) j-loop.
        qkv_subunit(MT_ORDER[0], 0)
        qkv_subunit(MT_ORDER[1], 0)

        # fillers[step] for the (hp0, ic0) block. mt0 chunk icx feeds key
        # tiles j in [4*icx, 4*icx+4) (S(j) is emitted at step j-1), mt4
        # chunk icx feeds query block icx (needed when ic0 ends). V(tt) must
        # precede O(tt) at step tt.
        ic0_fillers = {0: [(0, 1)], 1: [(0, 2)], 2: [(0, 3)],
                       4: [(1, 1)], 6: [(1, 2)], 8: [(1, 3)]}

        # mt tiles consumed as fillers during head-pair hp feed hp+1's S
        # matmuls, so they must fully drain before that hp block ends.
        qkv_pending = {hp: [(MT_ORDER[2 + 2 * hp], icx) for icx in range(4)]
                       + [(MT_ORDER[3 + 2 * hp], icx) for icx in range(4)]
                       for hp in range(3)}

        # ---- attention -----------------------------------------------
        def s_exp(hp, ic, j):
            kt = qk[hp]
            qt = qk[4 + hp]
            jsl = slice(j * 128, (j + 1) * 128)
            isl = slice(ic * 512, (ic + 1) * 512)
            sp = spx.tile([128, 1024], F32, tag="sp", name="sp")
            nc.tensor.matmul(sp[:, 0:512], kt[0:64, jsl], qt[0:64, isl],
                             start=True, stop=True)
            nc.tensor.matmul(sp[:, 512:1024], kt[64:128, jsl],
                             qt[64:128, isl], start=True, stop=True)
            ut = utp.tile([128, 1024], BF16, tag="ut", name="ut")
            nc.scalar.activation(ut, sp, EXP, scale=0.125)
            return ut

        proj_state = {}
        proj_q = []

        def proj_mm(hp, q2, u, ustep):
            # row-packed pair: head A weights at array rows 0:64, head B at
            # 64:128; separate psum tiles accumulating over all 16 u.
            if "ypA" not in proj_state:
                proj_state["ypA"] = ypx.tile([128, 512], F32, tag="yp",
                                             name="ypA")
                proj_state["ypB"] = ypx.tile([128, 512], F32, tag="yp",
                                             name="ypB")
            ypA, ypB = proj_state["ypA"], proj_state["ypB"]
            otr = ots[hp].rearrange("d (t u) -> d u t", u=16)
            csl = slice(q2 * 512, (q2 + 1) * 512)
            nc.tensor.matmul(ypA, otr[0:64, u, :], wp_sb[0:64, u, csl],
                             start=(ustep == 0), stop=(ustep == 15))
            nc.tensor.matmul(ypB, otr[64:128, u, :], wp_sb[64:128, u, csl],
                             start=(ustep == 0), stop=(ustep == 15))
            if ustep == 15:
                for h, yp in ((2 * hp, ypA), (2 * hp + 1, ypB)):
                    ysb = yps.tile([128, 512], F32, tag="ysb", name="ysb")
                    nc.vector.tensor_tensor(ysb, yp, pb_sb[:, csl], ADD)
                    nc.sync.dma_start(y[:, h, csl], ysb)
                proj_state.clear()

        # per-(hp,q2) count of emitted proj u-steps (for start/stop flags)
        proj_ucnt = {}

        def drain_proj():
            if not proj_q:
                return
            hp, q2, u = proj_q.pop(0)
            ustep = proj_ucnt.get((hp, q2), 0)
            proj_ucnt[(hp, q2)] = ustep + 1
            proj_mm(hp, q2, u, ustep)

        seq = [(hp, ic, j) for hp in range(4) for ic in range(4)
               for j in range(16)]
        tree = {}
        pending_norm = []

        def norm_tail(hp, ic, accf, osb):
            # PE-dependent part of the softmax normalization, deferred a
            # couple of j-steps so the in-order PE queue never waits on the
            # DVE tree chain. Denominators for A and B -> DRAM as one
            # [1024] row, reciprocal at 8 els/lane, broadcast back into the
            # matching partition halves.
            icsl = slice(ic * 512, (ic + 1) * 512)
            rsps = spx.tile([128, 1024], F32, tag="sp", name="rsps")
            nc.tensor.matmul(rsps[0:1, 0:512], ones128_sb,
                             accf[:, 0:512], start=True, stop=True)
            nc.tensor.matmul(rsps[32:33, 0:512], ones128_sb,
                             accf[:, 512:1024], start=True, stop=True,
                             tile_position=(0, 32))
            rsb = nrm.tile([33, 512], F32, tag="rsb", name="rsb")
            nc.vector.tensor_copy(rsb, rsps[0:33, 0:512])
            scr1 = dpool.tile([1024], F32, tag="scr1", name="scr1")
            nc.sync.dma_start(
                scr1.rearrange("(r f) -> r f", r=2), rsb[0:33:32, :])
            rst = nrm.tile([128, 8], F32, tag="rst", name="rst")
            nc.sync.dma_start(rst, scr1.rearrange("(p f) -> p f", f=8))
            nc.vector.reciprocal(rst, rst)
            scr2 = dpool.tile([1024], F32, tag="scr2", name="scr2")
            nc.sync.dma_start(scr2.rearrange("(p f) -> p f", f=8), rst)
            bcsb = nrm.tile([128, 512], F32, tag="bcsb", name="bcsb")
            nc.sync.dma_start(
                bcsb[0:64, :], scr2[None, 0:512].to_broadcast((64, 512)))
            nc.sync.dma_start(
                bcsb[64:128, :],
                scr2[None, 512:1024].to_broadcast((64, 512)))
            nc.vector.tensor_tensor(ots[hp][:, icsl], osb, bcsb, MULT)
            if ic == 3:
                proj_q.extend([(hp, 0, u) for u in range(16)])
                proj_q.extend([(hp, 1, u) for u in range(16)])

        pend = s_exp(*seq[0])
        optile = None
        for idx, (hp, ic, j) in enumerate(seq):
            hA, hB = 2 * hp, 2 * hp + 1
            if j == 0:
                optile = opx.tile([128, 512], F32, tag="op", name="op")
                if ic == 0 and hp <= 2:
                    # stream in the wt tiles for this hp's interleaved units
                    prefetch_wt(MT_ORDER[2 + 2 * hp])
                    prefetch_wt(MT_ORDER[3 + 2 * hp])
            nxt = s_exp(*seq[idx + 1]) if idx + 1 < len(seq) else None
            if j == 2 and pending_norm:
                norm_tail(*pending_norm.pop(0))
            # fillers that must precede or may overlap the O pair
            if hp == 0 and ic == 0:
                for mti, icx in ic0_fillers.get(j, ()):
                    qkv_subunit(MT_ORDER[mti], icx)
                v_unit(j)
            elif hp <= 2 and qkv_pending[hp] and (ic * 16 + j - 16) % 6 == 1:
                mt, icx = qkv_pending[hp].pop(0)
                qkv_subunit(mt, icx)
            # col-packed V pair: head A -> psum partitions 0:64, head B ->
            # 64:128, concurrent in the array
            nc.tensor.matmul(optile[0:64, :], vbuf[:, j, hA, :],
                             pend[:, 0:512], start=(j == 0), stop=(j == 15))
            nc.tensor.matmul(optile[64:128, :], vbuf[:, j, hB, :],
                             pend[:, 512:1024], start=(j == 0), stop=(j == 15),
                             tile_position=(0, 64))
            # denominator tree accumulation on DVE (bf16 4x mode)
            if j % 2 == 1:
                p = trp.tile([128, 1024], BF16, tag="tp", name="tp")
                nc.vector.tensor_tensor(p, tree.pop("u"), pend, ADD)
                tree[("p", (j // 2) % 2)] = p
            else:
                tree["u"] = pend
            if j % 4 == 3:
                qt_ = trp.tile([128, 1024], BF16, tag="tq", name="tq")
                nc.vector.tensor_tensor(qt_, tree.pop(("p", 0)),
                                        tree.pop(("p", 1)), ADD)
                tree[("q", (j // 4) % 2)] = qt_
            if j % 8 == 7:
                r = trp.tile([128, 1024], BF16, tag="tr", name="tr")
                nc.vector.tensor_tensor(r, tree.pop(("q", 0)),
                                        tree.pop(("q", 1)), ADD)
                tree[("r", j // 8)] = r
            pend = nxt
            drain_proj()
            if j == 15:
                accf = trp.tile([128, 1024], BF16, tag="ta", name="accf")
                nc.vector.tensor_tensor(accf, tree.pop(("r", 0)),
                                        tree.pop(("r", 1)), ADD)
                # copy the O psum out right away so the next ic's O chain
                # can start; normalize later on the SBUF copy.
                osb = nrm.tile([128, 512], F32, tag="osb", name="osb")
                nc.vector.tensor_copy(osb, optile)
                pending_norm.append((hp, ic, accf, osb))
        while pending_norm:
            norm_tail(*pending_norm.pop(0))
        while proj_q:
            drain_proj()

    nc.compile()
    return nc


def _in_maps(x, w_weight, w_bias, proj_weight, proj_bias):
    x = np.ascontiguousarray(x, np.float32)
    w_weight = np.ascontiguousarray(w_weight, np.float32)
    w_bias = np.ascontiguousarray(w_bias, np.float32)
    proj_weight = np.ascontiguousarray(proj_weight, np.float32)
    proj_bias = np.ascontiguousarray(proj_bias, np.float32)

    wpT = np.ascontiguousarray(
        proj_weight.T.reshape(16, 64, 1024).transpose(1, 0, 2).astype(ml_dtypes.bfloat16))
    pbr = np.ascontiguousarray(np.tile(proj_bias[None], (128, 1)))
    vones = np.ones((128, 1), dtype=ml_dtypes.bfloat16)

    maps = []
    for c in range(NCORES):
        b = c // 2
        h0 = (c % 2) * HPC
        xTc = np.ascontiguousarray(
            x[b].T.reshape(8, 128, 2048).transpose(1, 0, 2).astype(ml_dtypes.bfloat16))
        wk = w_weight[h0 * 64: h0 * 64 + 512]
        wq = w_weight[1024 + h0 * 64: 1024 + h0 * 64 + 512]
        wv = w_weight[2048 + h0 * 64: 2048 + h0 * 64 + 512]
        wqkvT = np.concatenate([wk.T, wq.T, wv.T], axis=1)  # [1024, 1536]
        wqkvT = np.ascontiguousarray(
            wqkvT.reshape(8, 128, 1536).transpose(1, 0, 2).astype(ml_dtypes.bfloat16))
        bk = w_bias[h0 * 64: h0 * 64 + 512]
        bq = w_bias[1024 + h0 * 64: 1024 + h0 * 64 + 512]
        bvc = w_bias[2048 + h0 * 64: 2048 + h0 * 64 + 512]
        bqkc = np.ascontiguousarray(
            np.concatenate([bk.reshape(4, 128).T, bq.reshape(4, 128).T], axis=1))
        bvr = np.ascontiguousarray(np.tile(bvc[None], (128, 1)))
        maps.append({
            "xT": xTc, "wqkv": wqkvT, "bqk": bqkc, "bv": bvr,
            "wp": wpT, "pb": pbr, "vones": vones,
        })
    return maps


def _install_ntff_hook():
    """Register the axon NTFF profiling hook (missing antenv.axon_hooks shim)."""
    import contextlib
    import ctypes
    import types

    if "antenv.axon_hooks" in sys.modules:
        return
    import antenv
    so_path = "/opt/axon/libaxon_pjrt.so"
    try:
        lib = ctypes.CDLL(so_path)
    except OSError:
        return
    if not hasattr(lib, "axon_start_nrt_profile"):
        return
    lib.axon_start_nrt_profile.argtypes = [ctypes.POINTER(ctypes.c_int64),
                                           ctypes.c_size_t]
    lib.axon_start_nrt_profile.restype = ctypes.c_int64
    lib.axon_stop_nrt_profile.argtypes = [ctypes.c_char_p]
    lib.axon_stop_nrt_profile.restype = ctypes.c_int64

    @contextlib.contextmanager
    def _hook(output_dir, device_ids):
        import jax
        jax.devices()
        if device_ids:
            ids = (ctypes.c_int64 * len(device_ids))(*device_ids)
            rc = lib.axon_start_nrt_profile(ids, len(device_ids))
        else:
            rc = lib.axon_start_nrt_profile(None, 0)
        if rc != 0:
            raise RuntimeError(f"axon_start_nrt_profile rc={rc}")
        try:
            yield
        finally:
            n = lib.axon_stop_nrt_profile(str(output_dir).encode())
            print(f"profile: {n} file(s) written to {output_dir}", file=sys.stderr)

    mod = types.ModuleType("antenv.axon_hooks")
    mod.get_axon_ntff_profile_hook = lambda: _hook
    mod.set_axon_ntff_profile_hook = lambda h: None
    sys.modules["antenv.axon_hooks"] = mod
    antenv.axon_hooks = mod


def _run(x, w_weight, w_bias, proj_weight, proj_bias, trace=False):
    from concourse.bass_utils import run_bass_kernel_spmd

    if trace:
        _install_ntff_hook()

    if "nc" not in _CACHE:
        _CACHE["nc"] = _build()
    nc = _CACHE["nc"]
    maps = _in_maps(x, w_weight, w_bias, proj_weight, proj_bias)
    res = run_bass_kernel_spmd(nc, maps, core_ids=list(range(NCORES)), trace=trace)
    out = np.zeros((B, T, C), np.float32)
    for c in range(NCORES):
        yc = res.results[c]["y"]  # [128, 8, 1024]
        b = c // 2
        h0 = (c % 2) * HPC
        for j in range(HPC):
            out[b, (h0 + j) * 128:(h0 + j + 1) * 128, :] = yc[:, j, :]
    return out, res.exec_time_ns


def kernel(x, w_weight, w_bias, proj_weight, proj_bias):
    out, _ = _run(x, w_weight, w_bias, proj_weight, proj_bias, trace=False)
    return out


def kernel_with_time(x, w_weight, w_bias, proj_weight, proj_bias):
    return _run(x, w_weight, w_bias, proj_weight, proj_bias, trace=True)


# revision 26
# speedup vs baseline: 1.6702x; 1.0132x over previous
"""Fused attention kernel for Trainium2, 8 NeuronCores.

Problem: B=4, T=2048, C=1024, nh=16, hs=64, fused QKV (chunk order k,q,v),
softmax attention, then (faithful reference bug) reshape (B,nh,T,hs)->(B,T,C)
directly before the output projection.

Key structural fact: with the buggy reshape, head h's attention output
occupies exactly rows [h*128, (h+1)*128) of the reshaped (T, C) matrix
(row tau = h*128 + t//16, col = (t%16)*64 + d). So everything after the
QKV projection is fully independent per (batch, head) pair; the output
projection needs no cross-head reduction.

Sharding: 8 cores = 4 batches x 2 head-groups (8 heads each). Each core
computes its batch's QKV slice and its 8 heads end-to-end. No collectives.

Schedule (v2): the Activation engine's exp stream (256 x [128,1024] tiles,
~1.04us each) is the pacing resource. Everything else is arranged so Act
starts early and never starves:
 - QKV is split into (mt, icx) subunits; only K0/Q0 run up front, the other
   six mt tiles are interleaved into the attention j-steps as PE fillers.
 - V tile units are emitted inside the first (hp0, ic0) j-loop just before
   the O matmul that consumes them.
 - Softmax denominators no longer stream exp tiles through the PE (the
   baseline's 512 ones-matmuls): the DVE tree-accumulates exp tiles in bf16
   (4x perf mode) and one gpsimd partition_all_reduce per (hp, ic) both
   reduces over keys and broadcasts across partitions. Reciprocal on DVE,
   normalization multiplies read the O psum directly.
 - Query blocks are strided (ic holds queries tau = ic mod 4) so each proj
   u-chunk (u = tau%16) depends on a single ic, letting the output
   projection drain during the attention stream instead of all at the end.
"""

import sys

import numpy as np

sys.path.insert(0, "/opt/trn_rl_repo")

import ml_dtypes  # noqa: E402

B, T, C = 4, 2048, 1024
NH, HS = 16, 64
NCORES = 8
HPC = 8  # heads per core

_CACHE = {}


def _build():
    from contextlib import ExitStack

    import concourse.bass as bass  # noqa: F401
    import concourse.bass_isa as bass_isa
    import concourse.mybir as mybir
    from concourse import bacc, library_config, tile

    F32 = mybir.dt.float32
    BF16 = mybir.dt.bfloat16
    ADD = mybir.AluOpType.add
    MULT = mybir.AluOpType.mult
    EXP = mybir.ActivationFunctionType.Exp

    nc = bacc.Bacc()
    vones = nc.dram_tensor("vones", [128, 1], BF16, kind="ExternalInput")
    xT = nc.dram_tensor("xT", [128, 8, 2048], BF16, kind="ExternalInput")
    wqkv = nc.dram_tensor("wqkv", [128, 8, 1536], BF16, kind="ExternalInput")
    bqk = nc.dram_tensor("bqk", [128, 8], F32, kind="ExternalInput")
    bv = nc.dram_tensor("bv", [128, 512], F32, kind="ExternalInput")
    wp = nc.dram_tensor("wp", [64, 16, 1024], BF16, kind="ExternalInput")
    pb = nc.dram_tensor("pb", [128, 1024], F32, kind="ExternalInput")
    y = nc.dram_tensor("y", [128, 8, 1024], F32, kind="ExternalOutput")

    MT_ORDER = [0, 4, 1, 5, 2, 6, 3, 7]

    with tile.TileContext(nc) as tc, ExitStack() as ctx:
        persist = ctx.enter_context(tc.tile_pool(name="persist", bufs=1))
        wsp = ctx.enter_context(tc.tile_pool(name="wstream", bufs=2))
        utp = ctx.enter_context(tc.tile_pool(name="utp", bufs=4))
        trp = ctx.enter_context(tc.tile_pool(name="treep", bufs=3))
        nrm = ctx.enter_context(tc.tile_pool(name="nrm", bufs=2))
        yps = ctx.enter_context(tc.tile_pool(name="ysb", bufs=2))

        # ---- persistent SBUF tensors + early DMAs --------------------
        bqk_sb = persist.tile([128, 8], F32, tag="bqk")
        nc.sync.dma_start(bqk_sb, bqk[:])
        bv_sb = persist.tile([128, 512], F32, tag="bv")
        nc.sync.dma_start(bv_sb, bv[:])
        pb_sb = persist.tile([128, 1024], F32, tag="pb")
        nc.sync.dma_start(pb_sb, pb[:])
        ones128_sb = persist.tile([128, 1], BF16, tag="ones128")
        nc.sync.dma_start(ones128_sb, vones[:])

        xts = persist.tile([128, 8, 2048], BF16, tag="xts")
        wt = {}

        def prefetch_wt(mt):
            wt[mt] = wsp.tile([128, 8, 128], BF16, tag="wt", name=f"wt{mt}")
            nc.sync.dma_start(wt[mt], wqkv[:, :, mt * 128:(mt + 1) * 128])

        prefetch_wt(MT_ORDER[0])
        prefetch_wt(MT_ORDER[1])
        nc.sync.dma_start(xts[:, :, 0:512], xT[:, :, 0:512])
        wv_sb = persist.tile([128, 8, 512], BF16, tag="wv")
        nc.sync.dma_start(wv_sb, wqkv[:, :, 1024:1536])
        for q in range(1, 4):
            nc.sync.dma_start(xts[:, :, q * 512:(q + 1) * 512],
                              xT[:, :, q * 512:(q + 1) * 512])
        wp_sb = persist.tile([128, 16, 1024], BF16, tag="wp")
        nc.sync.dma_start(wp_sb[0:64], wp[:])
        nc.sync.dma_start(wp_sb[64:128], wp[:])

        qk = [persist.tile([128, 2048], BF16, tag=f"qk{mt}", name=f"qk{mt}")
              for mt in range(8)]
        vbuf = persist.tile([128, 16, HPC, 64], BF16, tag="vbuf")
        ots = [persist.tile([128, 2048], BF16, tag=f"ot{hp}", name=f"ot{hp}")
               for hp in range(4)]

        # PSUM budget (8 banks): sp ring 2x[128,1024] = 4, O accumulate 1,
        # proj ypA+ypB = 2, QKV/V-unit accumulator 1. The per-(hp,ic)
        # denominator ones-matmul output borrows an sp-ring tile.
        spx = ctx.enter_context(tc.tile_pool(name="spool", bufs=2, space="PSUM"))
        opx = ctx.enter_context(tc.tile_pool(name="opool", bufs=1, space="PSUM"))
        ypx = ctx.enter_context(tc.tile_pool(name="ypool", bufs=2, space="PSUM"))
        aux = ctx.enter_context(tc.tile_pool(name="auxp", bufs=1, space="PSUM"))
        dpool = ctx.enter_context(tc.tile_pool(name="dpool", bufs=2,
                                               space="DRAM"))

        # ---- QKV / V unit emitters -----------------------------------
        # A subunit is 8 accumulating ct-matmuls + a bias add. Emitted as
        # individual "granules" so they interleave with the S/O stream
        # instead of blocking the in-order PE queue in 3-4us bursts.
        qkv_state = {}

        def qkv_granule(mt, icx, ct):
            if ct == 0:
                qkv_state[(mt, icx)] = aux.tile([128, 512], F32, tag="qkvps",
                                                name="qkvps")
            ps = qkv_state[(mt, icx)]
            isl = slice(icx * 512, (icx + 1) * 512)
            nc.tensor.matmul(ps, wt[mt][:, ct, :], xts[:, ct, isl],
                             start=(ct == 0), stop=(ct == 7))
            if ct == 7:
                nc.vector.tensor_tensor(
                    qk[mt][:, isl], ps,
                    bqk_sb[:, mt:mt + 1].to_broadcast((128, 512)), ADD)
                del qkv_state[(mt, icx)]

        def qkv_subunit(mt, icx):
            for ct in range(8):
                qkv_granule(mt, icx, ct)

        v_state = {}

        def v_granule(tt, ct):
            if ct == 0:
                v_state[tt] = aux.tile([128, 512], F32, tag="qkvps",
                                       name="vps")
            ps = v_state[tt]
            tsl = slice(tt * 128, (tt + 1) * 128)
            nc.tensor.matmul(ps, xts[:, ct, tsl], wv_sb[:, ct, :],
                             start=(ct == 0), stop=(ct == 7))
            if ct == 7:
                nc.vector.tensor_tensor(
                    vbuf[:, tt, :, :],
                    ps.rearrange("p (h d) -> p h d", d=64),
                    bv_sb.rearrange("p (h d) -> p h d", d=64), ADD)
                del v_state[tt]

        def v_unit(tt):
            for ct in range(8):
                v_granule(tt, ct)

        def granule(g):
            if g[0] == 'q':
                qkv_granule(*g[1:])
            else:
                v_granule(*g[1:])

        # Minimal warm-up: only the K0/Q0 chunks covering the first key
        # tiles and the first query block; everything else (6 subunits of
        # mt0/mt4, the V units) packs into the (hp0, ic0) j-loop.
        qkv_subunit(MT_ORDER[0], 0)
        qkv_subunit(MT_ORDER[1], 0)

        # Unified granule queue for the (hp0, ic0) block, drained 8 per
        # step. All groups share one psum ring buffer, so queue order IS
        # program order and groups never interleave. Order meets every
        # deadline: mt0 chunk icx feeds key tiles [4*icx, 4*icx+4) (S(j)
        # emitted at step j-1), V(tt) feeds O(tt) at step tt, mt4x1 feeds
        # query block ic1 (step 15).
        K0, Q0 = MT_ORDER[0], MT_ORDER[1]
        ic0_q = []
        for grp in ((('q', K0, 1),), (('v', 4),), (('q', K0, 2),),
                    (('v', 5),), (('v', 6),), (('q', K0, 3),),
                    (('v', 7),), (('v', 8),), (('q', Q0, 1),),
                    (('v', 9),), (('v', 10),), (('v', 11),), (('v', 12),),
                    (('v', 13),), (('v', 14),), (('v', 15),)):
            head = grp[0]
            for ct in range(8):
                ic0_q.append(head + (ct,))

        # mt tiles consumed as fillers during head-pair hp feed hp+1's S
        # matmuls, so they must fully drain before that hp block ends.
        # 3 granules per step drains them with plenty of margin.
        qkv_pending = {hp: [('q', MT_ORDER[2 + 2 * hp], icx, ct)
                            for icx in range(4) for ct in range(8)]
                       + [('q', MT_ORDER[3 + 2 * hp], icx, ct)
                          for icx in range(4) for ct in range(8)]
                       for hp in range(3)}
        # mt4's later query chunks are only needed when ic2/ic3 start
        qkv_pending[0] = [('q', Q0, 2, ct) for ct in range(8)] + \
                         [('q', Q0, 3, ct) for ct in range(8)] + \
                         qkv_pending[0]

        # ---- attention -----------------------------------------------
        def s_exp(hp, ic, j):
            kt = qk[hp]
            qt = qk[4 + hp]
            jsl = slice(j * 128, (j + 1) * 128)
            isl = slice(ic * 512, (ic + 1) * 512)
            sp = spx.tile([128, 1024], F32, tag="sp", name="sp")
            nc.tensor.matmul(sp[:, 0:512], kt[0:64, jsl], qt[0:64, isl],
                             start=True, stop=True)
            nc.tensor.matmul(sp[:, 512:1024], kt[64:128, jsl],
                             qt[64:128, isl], start=True, stop=True)
            ut = utp.tile([128, 1024], BF16, tag="ut", name="ut")
            nc.scalar.activation(ut, sp, EXP, scale=0.125)
            return ut

        proj_state = {}
        proj_q = []

        def proj_mm(hp, q2, u, ustep):
            # row-packed pair: head A weights at array rows 0:64, head B at
            # 64:128; separate psum tiles accumulating over all 16 u.
            if "ypA" not in proj_state:
                proj_state["ypA"] = ypx.tile([128, 512], F32, tag="yp",
                                             name="ypA")
                proj_state["ypB"] = ypx.tile([128, 512], F32, tag="yp",
                                             name="ypB")
            ypA, ypB = proj_state["ypA"], proj_state["ypB"]
            otr = ots[hp].rearrange("d (t u) -> d u t", u=16)
            csl = slice(q2 * 512, (q2 + 1) * 512)
            nc.tensor.matmul(ypA, otr[0:64, u, :], wp_sb[0:64, u, csl],
                             start=(ustep == 0), stop=(ustep == 15))
            nc.tensor.matmul(ypB, otr[64:128, u, :], wp_sb[64:128, u, csl],
                             start=(ustep == 0), stop=(ustep == 15))
            if ustep == 15:
                for h, yp in ((2 * hp, ypA), (2 * hp + 1, ypB)):
                    ysb = yps.tile([128, 512], F32, tag="ysb", name="ysb")
                    nc.vector.tensor_tensor(ysb, yp, pb_sb[:, csl], ADD)
                    nc.sync.dma_start(y[:, h, csl], ysb)
                proj_state.clear()

        # per-(hp,q2) count of emitted proj u-steps (for start/stop flags)
        proj_ucnt = {}

        def drain_proj():
            if not proj_q:
                return
            hp, q2, u = proj_q.pop(0)
            ustep = proj_ucnt.get((hp, q2), 0)
            proj_ucnt[(hp, q2)] = ustep + 1
            proj_mm(hp, q2, u, ustep)

        seq = [(hp, ic, j) for hp in range(4) for ic in range(4)
               for j in range(16)]
        tree = {}
        pending_norm = []

        def norm_tail(hp, ic, accf, osb):
            # PE-dependent part of the softmax normalization, deferred a
            # couple of j-steps so the in-order PE queue never waits on the
            # DVE tree chain. Denominators for A and B -> DRAM as one
            # [1024] row, reciprocal at 8 els/lane, broadcast back into the
            # matching partition halves.
            icsl = slice(ic * 512, (ic + 1) * 512)
            rsps = spx.tile([128, 1024], F32, tag="sp", name="rsps")
            nc.tensor.matmul(rsps[0:1, 0:512], ones128_sb,
                             accf[:, 0:512], start=True, stop=True)
            nc.tensor.matmul(rsps[32:33, 0:512], ones128_sb,
                             accf[:, 512:1024], start=True, stop=True,
                             tile_position=(0, 32))
            rsb = nrm.tile([33, 512], F32, tag="rsb", name="rsb")
            nc.vector.tensor_copy(rsb, rsps[0:33, 0:512])
            scr1 = dpool.tile([1024], F32, tag="scr1", name="scr1")
            nc.sync.dma_start(
                scr1.rearrange("(r f) -> r f", r=2), rsb[0:33:32, :])
            rst = nrm.tile([128, 8], F32, tag="rst", name="rst")
            nc.sync.dma_start(rst, scr1.rearrange("(p f) -> p f", f=8))
            nc.vector.reciprocal(rst, rst)
            scr2 = dpool.tile([1024], F32, tag="scr2", name="scr2")
            nc.sync.dma_start(scr2.rearrange("(p f) -> p f", f=8), rst)
            bcsb = nrm.tile([128, 512], F32, tag="bcsb", name="bcsb")
            nc.sync.dma_start(
                bcsb[0:64, :], scr2[None, 0:512].to_broadcast((64, 512)))
            nc.sync.dma_start(
                bcsb[64:128, :],
                scr2[None, 512:1024].to_broadcast((64, 512)))
            nc.vector.tensor_tensor(ots[hp][:, icsl], osb, bcsb, MULT)
            if ic == 3:
                proj_q.extend([(hp, 0, u) for u in range(16)])
                proj_q.extend([(hp, 1, u) for u in range(16)])

        pend = s_exp(*seq[0])
        # V0-3 run during the first exps instead of padding the first block
        for tt in range(4):
            v_unit(tt)
        optile = None
        for idx, (hp, ic, j) in enumerate(seq):
            hA, hB = 2 * hp, 2 * hp + 1
            if j == 0:
                optile = opx.tile([128, 512], F32, tag="op", name="op")
                if ic == 0 and hp <= 2:
                    # stream in the wt tiles for this hp's interleaved units
                    prefetch_wt(MT_ORDER[2 + 2 * hp])
                    prefetch_wt(MT_ORDER[3 + 2 * hp])
            nxt = s_exp(*seq[idx + 1]) if idx + 1 < len(seq) else None
            if j == 2 and pending_norm:
                norm_tail(*pending_norm.pop(0))
            # fillers that must precede or may overlap the O pair
            if hp == 0 and ic == 0:
                for _ in range(8):
                    if ic0_q:
                        granule(ic0_q.pop(0))
            elif hp <= 2 and qkv_pending[hp] and ic * 16 + j >= 16:
                for _ in range(3):
                    if qkv_pending[hp]:
                        granule(qkv_pending[hp].pop(0))
            # col-packed V pair: head A -> psum partitions 0:64, head B ->
            # 64:128, concurrent in the array
            nc.tensor.matmul(optile[0:64, :], vbuf[:, j, hA, :],
                             pend[:, 0:512], start=(j == 0), stop=(j == 15))
            nc.tensor.matmul(optile[64:128, :], vbuf[:, j, hB, :],
                             pend[:, 512:1024], start=(j == 0), stop=(j == 15),
                             tile_position=(0, 64))
            # denominator tree accumulation on DVE (bf16 4x mode)
            if j % 2 == 1:
                p = trp.tile([128, 1024], BF16, tag="tp", name="tp")
                nc.vector.tensor_tensor(p, tree.pop("u"), pend, ADD)
                tree[("p", (j // 2) % 2)] = p
            else:
                tree["u"] = pend
            if j % 4 == 3:
                qt_ = trp.tile([128, 1024], BF16, tag="tq", name="tq")
                nc.vector.tensor_tensor(qt_, tree.pop(("p", 0)),
                                        tree.pop(("p", 1)), ADD)
                tree[("q", (j // 4) % 2)] = qt_
            if j % 8 == 7:
                r = trp.tile([128, 1024], BF16, tag="tr", name="tr")
                nc.vector.tensor_tensor(r, tree.pop(("q", 0)),
                                        tree.pop(("q", 1)), ADD)
                tree[("r", j // 8)] = r
            pend = nxt
            drain_proj()
            if j == 15:
                accf = trp.tile([128, 1024], BF16, tag="ta", name="accf")
                nc.vector.tensor_tensor(accf, tree.pop(("r", 0)),
                                        tree.pop(("r", 1)), ADD)
                # copy the O psum out right away so the next ic's O chain
                # can start; normalize later on the SBUF copy.
                osb = nrm.tile([128, 512], F32, tag="osb", name="osb")
                nc.vector.tensor_copy(osb, optile)
                pending_norm.append((hp, ic, accf, osb))
        while pending_norm:
            norm_tail(*pending_norm.pop(0))
        while proj_q:
            drain_proj()

    nc.compile()
    return nc


def _in_maps(x, w_weight, w_bias, proj_weight, proj_bias):
    x = np.ascontiguousarray(x, np.float32)
    w_weight = np.ascontiguousarray(w_weight, np.float32)
    w_bias = np.ascontiguousarray(w_bias, np.float32)
    proj_weight = np.ascontiguousarray(proj_weight, np.float32)
    proj_bias = np.ascontiguousarray(proj_bias, np.float32)

    wpT = np.ascontiguousarray(
        proj_weight.T.reshape(16, 64, 1024).transpose(1, 0, 2).astype(ml_dtypes.bfloat16))
    pbr = np.ascontiguousarray(np.tile(proj_bias[None], (128, 1)))
    vones = np.ones((128, 1), dtype=ml_dtypes.bfloat16)

    maps = []
    for c in range(NCORES):
        b = c // 2
        h0 = (c % 2) * HPC
        xTc = np.ascontiguousarray(
            x[b].T.reshape(8, 128, 2048).transpose(1, 0, 2).astype(ml_dtypes.bfloat16))
        wk = w_weight[h0 * 64: h0 * 64 + 512]
        wq = w_weight[1024 + h0 * 64: 1024 + h0 * 64 + 512]
        wv = w_weight[2048 + h0 * 64: 2048 + h0 * 64 + 512]
        wqkvT = np.concatenate([wk.T, wq.T, wv.T], axis=1)  # [1024, 1536]
        wqkvT = np.ascontiguousarray(
            wqkvT.reshape(8, 128, 1536).transpose(1, 0, 2).astype(ml_dtypes.bfloat16))
        bk = w_bias[h0 * 64: h0 * 64 + 512]
        bq = w_bias[1024 + h0 * 64: 1024 + h0 * 64 + 512]
        bvc = w_bias[2048 + h0 * 64: 2048 + h0 * 64 + 512]
        bqkc = np.ascontiguousarray(
            np.concatenate([bk.reshape(4, 128).T, bq.reshape(4, 128).T], axis=1))
        bvr = np.ascontiguousarray(np.tile(bvc[None], (128, 1)))
        maps.append({
            "xT": xTc, "wqkv": wqkvT, "bqk": bqkc, "bv": bvr,
            "wp": wpT, "pb": pbr, "vones": vones,
        })
    return maps


def _install_ntff_hook():
    """Register the axon NTFF profiling hook (missing antenv.axon_hooks shim)."""
    import contextlib
    import ctypes
    import types

    if "antenv.axon_hooks" in sys.modules:
        return
    import antenv
    so_path = "/opt/axon/libaxon_pjrt.so"
    try:
        lib = ctypes.CDLL(so_path)
    except OSError:
        return
    if not hasattr(lib, "axon_start_nrt_profile"):
        return
    lib.axon_start_nrt_profile.argtypes = [ctypes.POINTER(ctypes.c_int64),
                                           ctypes.c_size_t]
    lib.axon_start_nrt_profile.restype = ctypes.c_int64
    lib.axon_stop_nrt_profile.argtypes = [ctypes.c_char_p]
    lib.axon_stop_nrt_profile.restype = ctypes.c_int64

    @contextlib.contextmanager
    def _hook(output_dir, device_ids):
        import jax
        jax.devices()
        if device_ids:
            ids = (ctypes.c_int64 * len(device_ids))(*device_ids)
            rc = lib.axon_start_nrt_profile(ids, len(device_ids))
        else:
            rc = lib.axon_start_nrt_profile(None, 0)
        if rc != 0:
            raise RuntimeError(f"axon_start_nrt_profile rc={rc}")
        try:
            yield
        finally:
            n = lib.axon_stop_nrt_profile(str(output_dir).encode())
            print(f"profile: {n} file(s) written to {output_dir}", file=sys.stderr)

    mod = types.ModuleType("antenv.axon_hooks")
    mod.get_axon_ntff_profile_hook = lambda: _hook
    mod.set_axon_ntff_profile_hook = lambda h: None
    sys.modules["antenv.axon_hooks"] = mod
    antenv.axon_hooks = mod


def _run(x, w_weight, w_bias, proj_weight, proj_bias, trace=False):
    from concourse.bass_utils import run_bass_kernel_spmd

    if trace:
        _install_ntff_hook()

    if "nc" not in _CACHE:
        _CACHE["nc"] = _build()
    nc = _CACHE["nc"]
    maps = _in_maps(x, w_weight, w_bias, proj_weight, proj_bias)
    res = run_bass_kernel_spmd(nc, maps, core_ids=list(range(NCORES)), trace=trace)
    out = np.zeros((B, T, C), np.float32)
    for c in range(NCORES):
        yc = res.results[c]["y"]  # [128, 8, 1024]
        b = c // 2
        h0 = (c % 2) * HPC
        for j in range(HPC):
            out[b, (h0 + j) * 128:(h0 + j + 1) * 128, :] = yc[:, j, :]
    return out, res.exec_time_ns


def kernel(x, w_weight, w_bias, proj_weight, proj_bias):
    out, _ = _run(x, w_weight, w_bias, proj_weight, proj_bias, trace=False)
    return out


def kernel_with_time(x, w_weight, w_bias, proj_weight, proj_bias):
    return _run(x, w_weight, w_bias, proj_weight, proj_bias, trace=True)


# revision 34
# speedup vs baseline: 1.6986x; 1.0170x over previous
"""Fused attention kernel for Trainium2, 8 NeuronCores.

Problem: B=4, T=2048, C=1024, nh=16, hs=64, fused QKV (chunk order k,q,v),
softmax attention, then (faithful reference bug) reshape (B,nh,T,hs)->(B,T,C)
directly before the output projection.

Key structural fact: with the buggy reshape, head h's attention output
occupies exactly rows [h*128, (h+1)*128) of the reshaped (T, C) matrix
(row tau = h*128 + t//16, col = (t%16)*64 + d). So everything after the
QKV projection is fully independent per (batch, head) pair; the output
projection needs no cross-head reduction.

Sharding: 8 cores = 4 batches x 2 head-groups (8 heads each). Each core
computes its batch's QKV slice and its 8 heads end-to-end. No collectives.

Schedule (v2): the Activation engine's exp stream (256 x [128,1024] tiles,
~1.04us each) is the pacing resource. Everything else is arranged so Act
starts early and never starves:
 - QKV is split into (mt, icx) subunits; only K0/Q0 run up front, the other
   six mt tiles are interleaved into the attention j-steps as PE fillers.
 - V tile units are emitted inside the first (hp0, ic0) j-loop just before
   the O matmul that consumes them.
 - Softmax denominators no longer stream exp tiles through the PE (the
   baseline's 512 ones-matmuls): the DVE tree-accumulates exp tiles in bf16
   (4x perf mode) and one gpsimd partition_all_reduce per (hp, ic) both
   reduces over keys and broadcasts across partitions. Reciprocal on DVE,
   normalization multiplies read the O psum directly.
 - Query blocks are strided (ic holds queries tau = ic mod 4) so each proj
   u-chunk (u = tau%16) depends on a single ic, letting the output
   projection drain during the attention stream instead of all at the end.
"""

import sys

import numpy as np

sys.path.insert(0, "/opt/trn_rl_repo")

import ml_dtypes  # noqa: E402

B, T, C = 4, 2048, 1024
NH, HS = 16, 64
NCORES = 8
HPC = 8  # heads per core

_CACHE = {}


def _build():
    from contextlib import ExitStack

    import concourse.bass as bass  # noqa: F401
    import concourse.bass_isa as bass_isa
    import concourse.mybir as mybir
    from concourse import bacc, library_config, tile

    F32 = mybir.dt.float32
    BF16 = mybir.dt.bfloat16
    ADD = mybir.AluOpType.add
    MULT = mybir.AluOpType.mult
    EXP = mybir.ActivationFunctionType.Exp

    nc = bacc.Bacc()
    vones = nc.dram_tensor("vones", [128, 1], BF16, kind="ExternalInput")
    xT = nc.dram_tensor("xT", [128, 8, 2048], BF16, kind="ExternalInput")
    wqkv = nc.dram_tensor("wqkv", [128, 8, 1536], BF16, kind="ExternalInput")
    bqk = nc.dram_tensor("bqk", [128, 8], F32, kind="ExternalInput")
    bv = nc.dram_tensor("bv", [128, 512], F32, kind="ExternalInput")
    wp = nc.dram_tensor("wp", [64, 16, 1024], BF16, kind="ExternalInput")
    pb = nc.dram_tensor("pb", [128, 1024], F32, kind="ExternalInput")
    y = nc.dram_tensor("y", [128, 8, 1024], F32, kind="ExternalOutput")

    MT_ORDER = [0, 4, 1, 5, 2, 6, 3, 7]

    with tile.TileContext(nc) as tc, ExitStack() as ctx:
        persist = ctx.enter_context(tc.tile_pool(name="persist", bufs=1))
        wsp = ctx.enter_context(tc.tile_pool(name="wstream", bufs=2))
        utp = ctx.enter_context(tc.tile_pool(name="utp", bufs=4))
        trp = ctx.enter_context(tc.tile_pool(name="treep", bufs=3))
        nrm = ctx.enter_context(tc.tile_pool(name="nrm", bufs=2))
        yps = ctx.enter_context(tc.tile_pool(name="ysb", bufs=2))

        # ---- persistent SBUF tensors + early DMAs --------------------
        # wt0/wt4 and the first x chunk gate the first matmul: issue them
        # before everything else on the queue.
        xts = persist.tile([128, 8, 2048], BF16, tag="xts")
        wt = {}

        def prefetch_wt(mt):
            wt[mt] = wsp.tile([128, 8, 128], BF16, tag="wt", name=f"wt{mt}")
            nc.sync.dma_start(wt[mt], wqkv[:, :, mt * 128:(mt + 1) * 128])

        prefetch_wt(MT_ORDER[0])
        prefetch_wt(MT_ORDER[1])
        nc.sync.dma_start(xts[:, :, 0:512], xT[:, :, 0:512])
        bqk_sb = persist.tile([128, 8], F32, tag="bqk")
        nc.sync.dma_start(bqk_sb, bqk[:])
        bv_sb = persist.tile([128, 512], F32, tag="bv")
        nc.sync.dma_start(bv_sb, bv[:])
        wv_sb = persist.tile([128, 8, 512], BF16, tag="wv")
        nc.sync.dma_start(wv_sb, wqkv[:, :, 1024:1536])
        for q in range(1, 4):
            nc.sync.dma_start(xts[:, :, q * 512:(q + 1) * 512],
                              xT[:, :, q * 512:(q + 1) * 512])
        pb_sb = persist.tile([128, 1024], F32, tag="pb")
        nc.sync.dma_start(pb_sb, pb[:])
        ones128_sb = persist.tile([128, 1], BF16, tag="ones128")
        nc.sync.dma_start(ones128_sb, vones[:])
        wp_sb = persist.tile([128, 16, 1024], BF16, tag="wp")
        nc.sync.dma_start(wp_sb[0:64], wp[:])
        nc.sync.dma_start(wp_sb[64:128], wp[:])

        qk = [persist.tile([128, 2048], BF16, tag=f"qk{mt}", name=f"qk{mt}")
              for mt in range(8)]
        vbuf = persist.tile([128, 16, HPC, 64], BF16, tag="vbuf")
        ots = [persist.tile([128, 2048], BF16, tag=f"ot{hp}", name=f"ot{hp}")
               for hp in range(4)]

        # PSUM budget (8 banks): sp ring 2x[128,1024] = 4, O accumulate 1,
        # proj ypA+ypB = 2, QKV/V-unit accumulator 1. The per-(hp,ic)
        # denominator ones-matmul output borrows an sp-ring tile.
        spx = ctx.enter_context(tc.tile_pool(name="spool", bufs=2, space="PSUM"))
        opx = ctx.enter_context(tc.tile_pool(name="opool", bufs=1, space="PSUM"))
        ypx = ctx.enter_context(tc.tile_pool(name="ypool", bufs=2, space="PSUM"))
        aux = ctx.enter_context(tc.tile_pool(name="auxp", bufs=1, space="PSUM"))
        dpool = ctx.enter_context(tc.tile_pool(name="dpool", bufs=2,
                                               space="DRAM"))

        # ---- QKV / V unit emitters -----------------------------------
        # A subunit is 8 accumulating ct-matmuls + a bias add. Emitted as
        # individual "granules" so they interleave with the S/O stream
        # instead of blocking the in-order PE queue in 3-4us bursts.
        qkv_state = {}

        def qkv_granule(mt, icx, ct):
            if ct == 0:
                qkv_state[(mt, icx)] = aux.tile([128, 512], F32, tag="qkvps",
                                                name="qkvps")
            ps = qkv_state[(mt, icx)]
            isl = slice(icx * 512, (icx + 1) * 512)
            nc.tensor.matmul(ps, wt[mt][:, ct, :], xts[:, ct, isl],
                             start=(ct == 0), stop=(ct == 7))
            if ct == 7:
                nc.vector.tensor_tensor(
                    qk[mt][:, isl], ps,
                    bqk_sb[:, mt:mt + 1].to_broadcast((128, 512)), ADD)
                del qkv_state[(mt, icx)]

        def qkv_subunit(mt, icx):
            for ct in range(8):
                qkv_granule(mt, icx, ct)

        v_state = {}

        def v_granule(tt, ct):
            if ct == 0:
                v_state[tt] = aux.tile([128, 512], F32, tag="qkvps",
                                       name="vps")
            ps = v_state[tt]
            tsl = slice(tt * 128, (tt + 1) * 128)
            nc.tensor.matmul(ps, xts[:, ct, tsl], wv_sb[:, ct, :],
                             start=(ct == 0), stop=(ct == 7))
            if ct == 7:
                nc.vector.tensor_tensor(
                    vbuf[:, tt, :, :],
                    ps.rearrange("p (h d) -> p h d", d=64),
                    bv_sb.rearrange("p (h d) -> p h d", d=64), ADD)
                del v_state[tt]

        def v_unit(tt):
            for ct in range(8):
                v_granule(tt, ct)

        def granule(g):
            if g[0] == 'q':
                qkv_granule(*g[1:])
            else:
                v_granule(*g[1:])

        # Minimal warm-up: only the K0/Q0 chunks covering the first key
        # tiles and the first query block; everything else (6 subunits of
        # mt0/mt4, the V units) packs into the (hp0, ic0) j-loop.
        qkv_subunit(MT_ORDER[0], 0)
        qkv_subunit(MT_ORDER[1], 0)

        # Unified granule queue for the (hp0, ic0) block, drained 9 per
        # step. All groups share one psum ring buffer, so queue order IS
        # program order and groups never interleave. Order meets every
        # deadline: mt0 chunk icx feeds key tiles [4*icx, 4*icx+4) (S(j)
        # emitted at step j-1), V(tt) feeds O(tt) at step tt, Q0x1 feeds
        # query block ic1 (step 15).
        K = [MT_ORDER[2 * i] for i in range(4)]
        Q = [MT_ORDER[2 * i + 1] for i in range(4)]
        ic0_groups = [('q', K[0], 1), ('v', 4), ('q', K[0], 2), ('v', 5),
                      ('v', 6), ('q', K[0], 3), ('v', 7), ('v', 8),
                      ('q', Q[0], 1), ('v', 9), ('v', 10), ('v', 11),
                      ('v', 12), ('v', 13), ('v', 14), ('v', 15)]
        ic0_q = [g + (ct,) for g in ic0_groups for ct in range(8)]

        # Deadline-balanced trickle queues: during head-pair hp's steps
        # 16..63, 2 granules/step feed the NEXT blocks' K/Q tiles. Q-chunk
        # icx of head-pair h is first needed at global step 64*h + 16*icx,
        # K tiles of h fully by step 64*h.
        def _g(mt, icx):
            return [('q', mt, icx, ct) for ct in range(8)]

        qkv_pending = {
            0: _g(Q[0], 2) + _g(Q[0], 3)
               + _g(K[1], 0) + _g(K[1], 1) + _g(K[1], 2) + _g(K[1], 3)
               + _g(Q[1], 0) + _g(Q[1], 1),
            1: _g(Q[1], 2) + _g(Q[1], 3)
               + _g(K[2], 0) + _g(K[2], 1) + _g(K[2], 2) + _g(K[2], 3)
               + _g(Q[2], 0) + _g(Q[2], 1),
            2: _g(Q[2], 2) + _g(Q[2], 3)
               + _g(K[3], 0) + _g(K[3], 1) + _g(K[3], 2) + _g(K[3], 3)
               + _g(Q[3], 0) + _g(Q[3], 1),
            3: _g(Q[3], 2) + _g(Q[3], 3),
        }

        # ---- attention -----------------------------------------------
        def s_exp(hp, ic, j):
            kt = qk[hp]
            qt = qk[4 + hp]
            jsl = slice(j * 128, (j + 1) * 128)
            isl = slice(ic * 512, (ic + 1) * 512)
            sp = spx.tile([128, 1024], F32, tag="sp", name="sp")
            nc.tensor.matmul(sp[:, 0:512], kt[0:64, jsl], qt[0:64, isl],
                             start=True, stop=True)
            nc.tensor.matmul(sp[:, 512:1024], kt[64:128, jsl],
                             qt[64:128, isl], start=True, stop=True)
            ut = utp.tile([128, 1024], BF16, tag="ut", name="ut")
            nc.scalar.activation(ut, sp, EXP, scale=0.125)
            return ut

        proj_state = {}
        proj_q = []

        def proj_mm(hp, q2, u, ustep):
            # row-packed pair: head A weights at array rows 0:64, head B at
            # 64:128; separate psum tiles accumulating over all 16 u.
            if "ypA" not in proj_state:
                proj_state["ypA"] = ypx.tile([128, 512], F32, tag="yp",
                                             name="ypA")
                proj_state["ypB"] = ypx.tile([128, 512], F32, tag="yp",
                                             name="ypB")
            ypA, ypB = proj_state["ypA"], proj_state["ypB"]
            otr = ots[hp].rearrange("d (t u) -> d u t", u=16)
            csl = slice(q2 * 512, (q2 + 1) * 512)
            nc.tensor.matmul(ypA, otr[0:64, u, :], wp_sb[0:64, u, csl],
                             start=(ustep == 0), stop=(ustep == 15))
            nc.tensor.matmul(ypB, otr[64:128, u, :], wp_sb[64:128, u, csl],
                             start=(ustep == 0), stop=(ustep == 15))
            if ustep == 15:
                for h, yp in ((2 * hp, ypA), (2 * hp + 1, ypB)):
                    ysb = yps.tile([128, 512], F32, tag="ysb", name="ysb")
                    nc.vector.tensor_tensor(ysb, yp, pb_sb[:, csl], ADD)
                    nc.sync.dma_start(y[:, h, csl], ysb)
                proj_state.clear()

        # per-(hp,q2) count of emitted proj u-steps (for start/stop flags)
        proj_ucnt = {}

        def drain_proj():
            if not proj_q:
                return
            hp, q2, u = proj_q.pop(0)
            ustep = proj_ucnt.get((hp, q2), 0)
            proj_ucnt[(hp, q2)] = ustep + 1
            proj_mm(hp, q2, u, ustep)

        seq = [(hp, ic, j) for hp in range(4) for ic in range(4)
               for j in range(16)]
        tree = {}
        pending_norm = []

        def norm_tail(hp, ic, accf, osb):
            # PE-dependent part of the softmax normalization, deferred a
            # couple of j-steps so the in-order PE queue never waits on the
            # DVE tree chain. Denominators for A and B -> DRAM as one
            # [1024] row, reciprocal at 8 els/lane, broadcast back into the
            # matching partition halves.
            icsl = slice(ic * 512, (ic + 1) * 512)
            rsps = spx.tile([128, 1024], F32, tag="sp", name="rsps")
            nc.tensor.matmul(rsps[0:1, 0:512], ones128_sb,
                             accf[:, 0:512], start=True, stop=True)
            nc.tensor.matmul(rsps[32:33, 0:512], ones128_sb,
                             accf[:, 512:1024], start=True, stop=True,
                             tile_position=(0, 32))
            rsb = nrm.tile([33, 512], F32, tag="rsb", name="rsb")
            nc.vector.tensor_copy(rsb, rsps[0:33, 0:512])
            scr1 = dpool.tile([1024], F32, tag="scr1", name="scr1")
            nc.sync.dma_start(
                scr1.rearrange("(r f) -> r f", r=2), rsb[0:33:32, :])
            rst = nrm.tile([128, 8], F32, tag="rst", name="rst")
            nc.sync.dma_start(rst, scr1.rearrange("(p f) -> p f", f=8))
            nc.vector.reciprocal(rst, rst)
            scr2 = dpool.tile([1024], F32, tag="scr2", name="scr2")
            nc.sync.dma_start(scr2.rearrange("(p f) -> p f", f=8), rst)
            bcsb = nrm.tile([128, 512], F32, tag="bcsb", name="bcsb")
            nc.sync.dma_start(
                bcsb[0:64, :], scr2[None, 0:512].to_broadcast((64, 512)))
            nc.sync.dma_start(
                bcsb[64:128, :],
                scr2[None, 512:1024].to_broadcast((64, 512)))
            nc.vector.tensor_tensor(ots[hp][:, icsl], osb, bcsb, MULT)
            if ic == 3:
                proj_q.extend([(hp, 0, u) for u in range(16)])
                proj_q.extend([(hp, 1, u) for u in range(16)])

        pend = s_exp(*seq[0])
        # V0-3 gate the first O matmuls; the rest stream in via ic0_q
        for tt in range(4):
            v_unit(tt)
        optile = None
        for idx, (hp, ic, j) in enumerate(seq):
            hA, hB = 2 * hp, 2 * hp + 1
            if j == 0:
                optile = opx.tile([128, 512], F32, tag="op", name="op")
                if ic == 0 and hp <= 2:
                    # stream in the wt tiles for this hp's interleaved units
                    prefetch_wt(MT_ORDER[2 + 2 * hp])
                    prefetch_wt(MT_ORDER[3 + 2 * hp])
            nxt = s_exp(*seq[idx + 1]) if idx + 1 < len(seq) else None
            if j == 2 and pending_norm:
                norm_tail(*pending_norm.pop(0))
            # fillers that must precede or may overlap the O pair
            if hp == 0 and ic == 0:
                for _ in range(8):
                    if ic0_q:
                        granule(ic0_q.pop(0))
            elif qkv_pending[hp] and ic * 16 + j >= 16:
                for _ in range(2):
                    if qkv_pending[hp]:
                        granule(qkv_pending[hp].pop(0))
            # col-packed V pair: head A -> psum partitions 0:64, head B ->
            # 64:128, concurrent in the array
            nc.tensor.matmul(optile[0:64, :], vbuf[:, j, hA, :],
                             pend[:, 0:512], start=(j == 0), stop=(j == 15))
            nc.tensor.matmul(optile[64:128, :], vbuf[:, j, hB, :],
                             pend[:, 512:1024], start=(j == 0), stop=(j == 15),
                             tile_position=(0, 64))
            # denominator tree accumulation on DVE (bf16 4x mode)
            if j % 2 == 1:
                p = trp.tile([128, 1024], BF16, tag="tp", name="tp")
                nc.vector.tensor_tensor(p, tree.pop("u"), pend, ADD)
                tree[("p", (j // 2) % 2)] = p
            else:
                tree["u"] = pend
            if j % 4 == 3:
                qt_ = trp.tile([128, 1024], BF16, tag="tq", name="tq")
                nc.vector.tensor_tensor(qt_, tree.pop(("p", 0)),
                                        tree.pop(("p", 1)), ADD)
                tree[("q", (j // 4) % 2)] = qt_
            if j % 8 == 7:
                r = trp.tile([128, 1024], BF16, tag="tr", name="tr")
                nc.vector.tensor_tensor(r, tree.pop(("q", 0)),
                                        tree.pop(("q", 1)), ADD)
                tree[("r", j // 8)] = r
            pend = nxt
            drain_proj()
            if len(proj_q) > 16:
                drain_proj()
            if j == 15:
                accf = trp.tile([128, 1024], BF16, tag="ta", name="accf")
                nc.vector.tensor_tensor(accf, tree.pop(("r", 0)),
                                        tree.pop(("r", 1)), ADD)
                # copy the O psum out right away so the next ic's O chain
                # can start; normalize later on the SBUF copy.
                osb = nrm.tile([128, 512], F32, tag="osb", name="osb")
                nc.vector.tensor_copy(osb, optile)
                pending_norm.append((hp, ic, accf, osb))
        while pending_norm:
            norm_tail(*pending_norm.pop(0))
        while proj_q:
            drain_proj()

    nc.compile()
    return nc


def _in_maps(x, w_weight, w_bias, proj_weight, proj_bias):
    x = np.ascontiguousarray(x, np.float32)
    w_weight = np.ascontiguousarray(w_weight, np.float32)
    w_bias = np.ascontiguousarray(w_bias, np.float32)
    proj_weight = np.ascontiguousarray(proj_weight, np.float32)
    proj_bias = np.ascontiguousarray(proj_bias, np.float32)

    wpT = np.ascontiguousarray(
        proj_weight.T.reshape(16, 64, 1024).transpose(1, 0, 2).astype(ml_dtypes.bfloat16))
    pbr = np.ascontiguousarray(np.tile(proj_bias[None], (128, 1)))
    vones = np.ones((128, 1), dtype=ml_dtypes.bfloat16)

    maps = []
    for c in range(NCORES):
        b = c // 2
        h0 = (c % 2) * HPC
        xTc = np.ascontiguousarray(
            x[b].T.reshape(8, 128, 2048).transpose(1, 0, 2).astype(ml_dtypes.bfloat16))
        wk = w_weight[h0 * 64: h0 * 64 + 512]
        wq = w_weight[1024 + h0 * 64: 1024 + h0 * 64 + 512]
        wv = w_weight[2048 + h0 * 64: 2048 + h0 * 64 + 512]
        wqkvT = np.concatenate([wk.T, wq.T, wv.T], axis=1)  # [1024, 1536]
        wqkvT = np.ascontiguousarray(
            wqkvT.reshape(8, 128, 1536).transpose(1, 0, 2).astype(ml_dtypes.bfloat16))
        bk = w_bias[h0 * 64: h0 * 64 + 512]
        bq = w_bias[1024 + h0 * 64: 1024 + h0 * 64 + 512]
        bvc = w_bias[2048 + h0 * 64: 2048 + h0 * 64 + 512]
        bqkc = np.ascontiguousarray(
            np.concatenate([bk.reshape(4, 128).T, bq.reshape(4, 128).T], axis=1))
        bvr = np.ascontiguousarray(np.tile(bvc[None], (128, 1)))
        maps.append({
            "xT": xTc, "wqkv": wqkvT, "bqk": bqkc, "bv": bvr,
            "wp": wpT, "pb": pbr, "vones": vones,
        })
    return maps


def _install_ntff_hook():
    """Register the axon NTFF profiling hook (missing antenv.axon_hooks shim)."""
    import contextlib
    import ctypes
    import types

    if "antenv.axon_hooks" in sys.modules:
        return
    import antenv
    so_path = "/opt/axon/libaxon_pjrt.so"
    try:
        lib = ctypes.CDLL(so_path)
    except OSError:
        return
    if not hasattr(lib, "axon_start_nrt_profile"):
        return
    lib.axon_start_nrt_profile.argtypes = [ctypes.POINTER(ctypes.c_int64),
                                           ctypes.c_size_t]
    lib.axon_start_nrt_profile.restype = ctypes.c_int64
    lib.axon_stop_nrt_profile.argtypes = [ctypes.c_char_p]
    lib.axon_stop_nrt_profile.restype = ctypes.c_int64

    @contextlib.contextmanager
    def _hook(output_dir, device_ids):
        import jax
        jax.devices()
        if device_ids:
            ids = (ctypes.c_int64 * len(device_ids))(*device_ids)
            rc = lib.axon_start_nrt_profile(ids, len(device_ids))
        else:
            rc = lib.axon_start_nrt_profile(None, 0)
        if rc != 0:
            raise RuntimeError(f"axon_start_nrt_profile rc={rc}")
        try:
            yield
        finally:
            n = lib.axon_stop_nrt_profile(str(output_dir).encode())
            print(f"profile: {n} file(s) written to {output_dir}", file=sys.stderr)

    mod = types.ModuleType("antenv.axon_hooks")
    mod.get_axon_ntff_profile_hook = lambda: _hook
    mod.set_axon_ntff_profile_hook = lambda h: None
    sys.modules["antenv.axon_hooks"] = mod
    antenv.axon_hooks = mod


def _run(x, w_weight, w_bias, proj_weight, proj_bias, trace=False):
    from concourse.bass_utils import run_bass_kernel_spmd

    if trace:
        _install_ntff_hook()

    if "nc" not in _CACHE:
        _CACHE["nc"] = _build()
    nc = _CACHE["nc"]
    maps = _in_maps(x, w_weight, w_bias, proj_weight, proj_bias)
    res = run_bass_kernel_spmd(nc, maps, core_ids=list(range(NCORES)), trace=trace)
    out = np.zeros((B, T, C), np.float32)
    for c in range(NCORES):
        yc = res.results[c]["y"]  # [128, 8, 1024]
        b = c // 2
        h0 = (c % 2) * HPC
        for j in range(HPC):
            out[b, (h0 + j) * 128:(h0 + j + 1) * 128, :] = yc[:, j, :]
    return out, res.exec_time_ns


def kernel(x, w_weight, w_bias, proj_weight, proj_bias):
    out, _ = _run(x, w_weight, w_bias, proj_weight, proj_bias, trace=False)
    return out


def kernel_with_time(x, w_weight, w_bias, proj_weight, proj_bias):
    return _run(x, w_weight, w_bias, proj_weight, proj_bias, trace=True)
